# revision 1
# baseline (speedup 1.0000x reference)
"""HSA (hierarchical splat attention) Bass kernel for Trainium2, 8 NeuronCores.

Math (per batch b):
    q = query @ Wq.T + bq                      [S, D]
    v = value @ Wv.T + bv                      [S, D]
    d2[s,n]  = |q_s|^2 - 2 q_s.c_n + |c_n|^2
    G[s,n]   = exp(-d2[s,n] * inv2v[n]),  inv2v = 0.5*exp(-2*log_scales)
    Asym[s,t]= sum_n G[s,n]*amp[n]*G[t,n]      (symmetric!)
    A        = Asym / (rowsum(Asym) + eps)
    out      = A @ v ;  y = out @ Wo.T + bo

Sharding: 8 cores = (batch b = c//2, seq-half h = c%2). Each core computes the
full-batch q-projection/G/v (needed for its rows of A) and its own 1024 output
rows. No collectives. Host pre-transposes inputs so every matmul has its
natural lhsT/rhs layout; the sequence axis is rolled per-core so "own" rows are
always columns 0..1023 (valid since A@v and rowsum are permutation-invariant
over t, and the q-side order is rolled consistently).

Device dataflow (all matmuls are lhsT.T @ rhs, K on partitions):
  qT[e,s]   : lhsT=Wq.T chunk,  rhs=xqT chunk           (accum over d)
  d2T[n,s]  : lhsT=(-2C).T,     rhs=qT   (+ ones64 lhsT, rhs=qT^2 -> |q|^2)
  GT,GampT  : ACT exp with per-partition scale=-inv2v, bias=-inv2v*c2 (+ln amp)
  v[t,e]    : lhsT=xvT chunk,   rhs=Wv.T chunk          (accum over d)
  AsymT[t,s]: lhsT=GT t-chunk,  rhs=GampT own-s   (K=64, one shot)
  rs[s]     : lhsT=ones128,     rhs=AsymT               (accum over t)
  outT[d,s] : lhsT=v d-slice,   rhs=AsymT               (accum over t)
  normalize : outT *= 1/(rs+eps)   (free-dim broadcast tiles)
  y[s,e]    : lhsT=outT s-slice, rhs=Wo.T chunk + bo    (accum over d)
"""

import numpy as np
import ml_dtypes

BF16 = ml_dtypes.bfloat16
EMBED = 1024
S = 2048
NSPL = 64
B = 4
NCORES = 8
P = 128
KC = EMBED // P   # 8 contraction chunks over d/e
TCH = S // P      # 16 t-chunks
SOWN = S // 2     # 1024 own output rows per core
SCH = SOWN // P   # 8
EPS = 1e-8

_PROG = None  # cached (nc, input_names)


def _build_program():
    import concourse.bass as bass
    import concourse.mybir as mybir
    from concourse import bacc
    from concourse.tile import TileContext
    from concourse.bass import ts, ds

    f32 = mybir.dt.float32
    bf16 = mybir.dt.bfloat16
    AF = mybir.ActivationFunctionType

    nc = bacc.Bacc("TRN2", target_bir_lowering=False, debug=False)
    xqT = nc.declare_dram_parameter("xqT", [EMBED, S], bf16, isOutput=False)
    xvT = nc.declare_dram_parameter("xvT", [EMBED, S], bf16, isOutput=False)
    wqT = nc.declare_dram_parameter("wqT", [EMBED, EMBED], bf16, isOutput=False)
    wvT = nc.declare_dram_parameter("wvT", [EMBED, EMBED], bf16, isOutput=False)
    woT = nc.declare_dram_parameter("woT", [EMBED, EMBED], bf16, isOutput=False)
    ctm2 = nc.declare_dram_parameter("ctm2", [EMBED, NSPL], bf16, isOutput=False)
    bq2 = nc.declare_dram_parameter("bq2", [P, KC], f32, isOutput=False)
    bvb = nc.declare_dram_parameter("bvb", [P, EMBED], f32, isOutput=False)
    bob = nc.declare_dram_parameter("bob", [P, EMBED], f32, isOutput=False)
    scn = nc.declare_dram_parameter("scn", [NSPL, 1], f32, isOutput=False)
    bgn = nc.declare_dram_parameter("bgn", [NSPL, 1], f32, isOutput=False)
    bgan = nc.declare_dram_parameter("bgan", [NSPL, 1], f32, isOutput=False)
    one64 = nc.declare_dram_parameter("one64", [P, NSPL], bf16, isOutput=False)
    one128 = nc.declare_dram_parameter("one128", [P, P], bf16, isOutput=False)
    y = nc.declare_dram_parameter("y", [SOWN, EMBED], f32, isOutput=True)

    with TileContext(nc) as tc:
        cpool_cm = tc.tile_pool(name="const", bufs=1)
        cpool = cpool_cm.__enter__()
        bq_sb = cpool.tile([P, KC], f32)
        bv_sb = cpool.tile([P, EMBED], f32)
        bo_sb = cpool.tile([P, EMBED], f32)
        sc_sb = cpool.tile([NSPL, 1], f32)
        bg_sb = cpool.tile([NSPL, 1], f32)
        bga_sb = cpool.tile([NSPL, 1], f32)
        o64_sb = cpool.tile([P, NSPL], bf16)
        o128_sb = cpool.tile([P, P], bf16)
        ct_sb = cpool.tile([P, KC, NSPL], bf16)
        gt = cpool.tile([NSPL, S], bf16)
        gamp = cpool.tile([NSPL, SOWN], bf16)

        nc.sync.dma_start(bq_sb[:], bq2[:])
        nc.sync.dma_start(sc_sb[:], scn[:])
        nc.sync.dma_start(bg_sb[:], bgn[:])
        nc.sync.dma_start(bga_sb[:], bgan[:])
        nc.sync.dma_start(o64_sb[:], one64[:])
        nc.sync.dma_start(o128_sb[:], one128[:])
        ctr = ctm2.rearrange("(k p) n -> k p n", p=P)
        for k in range(KC):
            nc.sync.dma_start(ct_sb[:, k], ctr[k])

        # ---------------- Phase A: q projection + G ----------------
        with tc.tile_pool(name="pa", bufs=1) as pa, \
             tc.tile_pool(name="qe", bufs=3) as qep, \
             tc.tile_pool(name="sqe", bufs=3) as sqp, \
             tc.tile_pool(name="psq", bufs=4, space="PSUM") as psq, \
             tc.tile_pool(name="psd2", bufs=4, space="PSUM") as psd2:
            xq = pa.tile([P, KC, S], bf16)
            wq = pa.tile([P, KC, EMBED], bf16)
            wqr = wqT.rearrange("(k p) e -> k p e", p=P)
            xqr = xqT.rearrange("(k p) s -> k p s", p=P)
            for k in range(KC):
                nc.sync.dma_start(wq[:, k], wqr[k])
                nc.sync.dma_start(xq[:, k], xqr[k])
            nc.sync.dma_start(bv_sb[:], bvb[:])
            nc.sync.dma_start(bo_sb[:], bob[:])
            d2ps = [psd2.tile([NSPL, 512], f32, tag="d2", name=f"d2ps{i}") for i in range(4)]
            for e in range(KC):
                qps = [psq.tile([P, 512], f32, tag="qps", name=f"qps{e}_{i}") for i in range(4)]
                for k in range(KC):
                    for s4 in range(4):
                        nc.tensor.matmul(
                            qps[s4], wq[:, k, ts(e, P)], xq[:, k, ts(s4, 512)],
                            start=(k == 0), stop=(k == KC - 1))
                qe = qep.tile([P, S], bf16, tag="qe")
                for s4 in range(4):
                    if s4 % 2 == 0:
                        nc.scalar.activation(qe[:, ts(s4, 512)], qps[s4],
                                             AF.Identity, bias=bq_sb[:, ds(e, 1)])
                    else:
                        nc.vector.tensor_scalar_add(qe[:, ts(s4, 512)], qps[s4],
                                                    bq_sb[:, ds(e, 1)])
                sq = sqp.tile([P, S], bf16, tag="sq")
                nc.vector.tensor_mul(sq, qe, qe)
                for s4 in range(4):
                    nc.tensor.matmul(d2ps[s4], ct_sb[:, e], qe[:, ts(s4, 512)],
                                     start=(e == 0), stop=False)
                for s4 in range(4):
                    nc.tensor.matmul(d2ps[s4], o64_sb[:], sq[:, ts(s4, 512)],
                                     start=False, stop=(e == KC - 1))
            for s4 in range(4):
                nc.scalar.activation(gt[:, ts(s4, 512)], d2ps[s4], AF.Exp,
                                     bias=bg_sb[:], scale=sc_sb[:])
            for s2 in range(2):
                nc.scalar.activation(gamp[:, ts(s2, 512)], d2ps[s2], AF.Exp,
                                     bias=bga_sb[:], scale=sc_sb[:])

        # ---------------- Phase B: v projection ----------------
        vpool_cm = tc.tile_pool(name="vpool", bufs=1)
        vpool = vpool_cm.__enter__()
        v_sb = vpool.tile([P, TCH, EMBED], bf16)
        with tc.tile_pool(name="pb", bufs=1) as pb, \
             tc.tile_pool(name="psv", bufs=3, space="PSUM") as psv:
            xv = pb.tile([P, KC, S], bf16)
            wv = pb.tile([P, KC, EMBED], bf16)
            wvr = wvT.rearrange("(k p) e -> k p e", p=P)
            xvr = xvT.rearrange("(k p) s -> k p s", p=P)
            for k in range(KC):
                nc.sync.dma_start(wv[:, k], wvr[k])
                nc.sync.dma_start(xv[:, k], xvr[k])
            for t in range(TCH):
                vps = psv.tile([P, EMBED], f32, tag="vps")
                for k in range(KC):
                    for eh in range(2):
                        nc.tensor.matmul(
                            vps[:, ts(eh, 512)], xv[:, k, ts(t, P)],
                            wv[:, k, ts(eh, 512)],
                            start=(k == 0), stop=(k == KC - 1))
                nc.vector.tensor_add(v_sb[:, t], vps, bv_sb)

        # ---------------- Phase C+D fused: Asym, rowsum, outT ----------------
        wpool_cm = tc.tile_pool(name="wpool", bufs=1)
        wpool = wpool_cm.__enter__()
        wo = wpool.tile([P, KC, EMBED], bf16)
        wor = woT.rearrange("(k p) e -> k p e", p=P)
        for k in range(KC):
            nc.sync.dma_start(wo[:, k], wor[k])
        otpool_cm = tc.tile_pool(name="otpool", bufs=1)
        otpool = otpool_cm.__enter__()
        outT = otpool.tile([P, KC, SOWN], bf16)

        with tc.tile_pool(name="asym", bufs=4) as asp, \
             tc.tile_pool(name="rssb", bufs=2) as rsp, \
             tc.tile_pool(name="psas", bufs=2, space="PSUM") as psas, \
             tc.tile_pool(name="pso", bufs=4, space="PSUM") as pso, \
             tc.tile_pool(name="psrs", bufs=1, space="PSUM") as psrs:
            for st in range(2):          # own-s tiles of 512
                rsps = psrs.tile([P, 512], f32, tag="rs")
                rsin = None
                for dh in range(2):      # d-chunk halves (4 each)
                    ops = [pso.tile([P, 512], f32, tag="ops", name=f"ops{st}_{dh}_{i}") for i in range(4)]
                    for t in range(TCH):
                        aps = psas.tile([P, 512], f32, tag="aps")
                        nc.tensor.matmul(aps, gt[:, ts(t, P)],
                                         gamp[:, ts(st, 512)],
                                         start=True, stop=True)
                        asy = asp.tile([P, 512], bf16, tag="asy")
                        if t % 2 == 0:
                            nc.vector.tensor_copy(asy, aps)
                        else:
                            nc.scalar.activation(asy, aps, AF.Copy)
                        if dh == 0:
                            nc.tensor.matmul(rsps, o128_sb[:], asy,
                                             start=(t == 0), stop=(t == TCH - 1))
                        for i in range(4):
                            d = dh * 4 + i
                            nc.tensor.matmul(ops[i], v_sb[:, t, ts(d, P)], asy,
                                             start=(t == 0), stop=(t == TCH - 1))
                    if dh == 0:
                        rs_sb = rsp.tile([P, 512], f32, tag="rss")
                        nc.vector.tensor_scalar_add(rs_sb, rsps, EPS)
                        rsin = rsp.tile([P, 512], f32, tag="rsin")
                        nc.vector.reciprocal(rsin, rs_sb)
                    for i in range(4):
                        d = dh * 4 + i
                        nc.vector.tensor_mul(outT[:, d, ds(st * 512, 512)],
                                             ops[i], rsin)

        # ---------------- Phase E: output projection ----------------
        with tc.tile_pool(name="ybuf", bufs=2) as yb, \
             tc.tile_pool(name="psy", bufs=3, space="PSUM") as psy:
            yr = y.rearrange("(c p) e -> c p e", p=P)
            for sc in range(SCH):
                yps = psy.tile([P, EMBED], f32, tag="yps")
                for k in range(KC):
                    for eh in range(2):
                        nc.tensor.matmul(
                            yps[:, ts(eh, 512)], outT[:, k, ts(sc, P)],
                            wo[:, k, ts(eh, 512)],
                            start=(k == 0), stop=(k == KC - 1))
                ysb = yb.tile([P, EMBED], f32, tag="ysb")
                nc.vector.tensor_add(ysb, yps, bo_sb)
                nc.sync.dma_start(yr[sc], ysb)
        otpool_cm.__exit__(None, None, None)
        wpool_cm.__exit__(None, None, None)
        vpool_cm.__exit__(None, None, None)
        cpool_cm.__exit__(None, None, None)

    nc.finalize()
    return nc


def _prep_inputs(query, key, value, Wq, bq, Wk, bk, Wv, bv, Wo, bo,
                 splat_centers, splat_log_scales, splat_amplitudes):
    """Build the 8 per-core input maps (host-side sharding/layout prep)."""
    f = np.float32
    q = np.asarray(query, f)
    v = np.asarray(value, f)
    Wq = np.asarray(Wq, f); bq = np.asarray(bq, f)
    Wv = np.asarray(Wv, f); bv = np.asarray(bv, f)
    Wo = np.asarray(Wo, f); bo = np.asarray(bo, f)
    C = np.asarray(splat_centers, f)
    ls = np.asarray(splat_log_scales, f)
    amp = np.asarray(splat_amplitudes, f)

    wqT = np.ascontiguousarray(Wq.T).astype(BF16)
    wvT = np.ascontiguousarray(Wv.T).astype(BF16)
    woT = np.ascontiguousarray(Wo.T).astype(BF16)
    ctm2 = np.ascontiguousarray((-2.0 * C).T).astype(BF16)
    bq2 = np.ascontiguousarray(bq.reshape(KC, P).T)
    bvb = np.ascontiguousarray(np.broadcast_to(bv, (P, EMBED)))
    bob = np.ascontiguousarray(np.broadcast_to(bo, (P, EMBED)))
    inv2v = 0.5 * np.exp(-2.0 * ls).astype(f)
    c2 = (C.astype(np.float64) ** 2).sum(1)
    scn = (-inv2v).reshape(NSPL, 1).astype(f)
    bgn = (-inv2v * c2).reshape(NSPL, 1).astype(f)
    # fold amplitude into one G factor: amp*exp(x) = exp(x + ln amp)
    bgan = (-inv2v * c2 + np.log(np.maximum(amp, 1e-38))).reshape(NSPL, 1).astype(f)
    one64 = np.ones((P, NSPL), BF16)
    one128 = np.ones((P, P), BF16)

    shared = dict(wqT=wqT, wvT=wvT, woT=woT, ctm2=ctm2, bq2=bq2, bvb=bvb,
                  bob=bob, scn=scn, bgn=bgn, bgan=bgan, one64=one64,
                  one128=one128)
    in_maps = []
    for c in range(NCORES):
        b, h = c // 2, c % 2
        # roll the sequence axis so own rows are always 0..1023
        qb = np.concatenate([q[b, h * SOWN:], q[b, :h * SOWN]], axis=0)
        vb = np.concatenate([v[b, h * SOWN:], v[b, :h * SOWN]], axis=0)
        m = dict(shared)
        m["xqT"] = np.ascontiguousarray(qb.T).astype(BF16)
        m["xvT"] = np.ascontiguousarray(vb.T).astype(BF16)
        in_maps.append(m)
    return in_maps


def run_cores(inputs, trace=False):
    """Run the SPMD kernel; returns (full_output, BassKernelResults)."""
    global _PROG
    from concourse.bass_utils import run_bass_kernel_spmd
    if _PROG is None:
        _PROG = _build_program()
    nc = _PROG
    in_maps = _prep_inputs(**inputs)
    res = run_bass_kernel_spmd(nc, in_maps, list(range(NCORES)), trace=trace)
    out = np.empty((B, S, EMBED), np.float32)
    for c in range(NCORES):
        b, h = c // 2, c % 2
        out[b, h * SOWN:(h + 1) * SOWN] = res.results[c]["y"]
    return out, res


def kernel(**inputs):
    out, _ = run_cores(inputs, trace=False)
    return out



# revision 6
# speedup vs baseline: 1.2816x; 1.2816x over previous
"""HSA (hierarchical splat attention) Bass kernel for Trainium2, 8 NeuronCores.

Math (per batch b):
    q = query @ Wq.T + bq                      [S, D]
    v = value @ Wv.T + bv                      [S, D]
    d2[s,n]  = |q_s|^2 - 2 q_s.c_n + |c_n|^2
    G[s,n]   = exp(-d2[s,n] * inv2v[n]),  inv2v = 0.5*exp(-2*log_scales)
    Asym[s,t]= sum_n G[s,n]*amp[n]*G[t,n]      (rank-N_SPLATS!)
    A        = Asym / (rowsum(Asym) + eps)
    out      = A @ v ;  y = out @ Wo.T + bo

Key identity: with G' = G * sqrt(amp),  Asym = G' G'^T, so
    out = G' @ M / rs,   M  = G'^T @ V          [N, D]   (never form [S,S])
    rs[s] = G'[s,:] . gsum + eps,  gsum = G'^T @ 1

Sharding: core c = (batch b = c//2, seq-half h = c%2). Each core receives ONLY
its own 1024-token halves of query/value (contiguous f32 views - zero host
prep), projects them on-device (PE-transpose for layout), computes its partial
M/gsum, and a single pair-wise AllReduce of [64,1026] f32 completes the
t-contraction. Weights/constants are content-hash cached device-resident
arrays, so steady-state host->device traffic is just q,v halves + y out.
"""

import numpy as np
import ml_dtypes

BF16 = ml_dtypes.bfloat16
EMBED = 1024
S = 2048
NSPL = 64
B = 4
NCORES = 8
P = 128
KC = EMBED // P   # 8 chunks over d/e
SOWN = S // 2     # 1024 own tokens per core
SCH = SOWN // P   # 8 own s/t chunks
MW = EMBED + 2    # M payload width: 1024 d-cols + 2 gsum half-cols
EPS = 1e-8

_PROG = None       # cached bass program
_DISPATCH = None   # cached jit etc.
_CONSTS = None     # cached (key, host_arrays, device_arrays)


def _build_program():
    import concourse.mybir as mybir
    from concourse import bacc
    from concourse.tile import TileContext
    from concourse.bass import ts, ds

    f32 = mybir.dt.float32
    bf16 = mybir.dt.bfloat16
    AF = mybir.ActivationFunctionType

    nc = bacc.Bacc("TRN2", target_bir_lowering=False, debug=False,
                   num_devices=NCORES)
    xq = nc.declare_dram_parameter("xq", [SOWN, EMBED], f32, isOutput=False)
    xv = nc.declare_dram_parameter("xv", [SOWN, EMBED], f32, isOutput=False)
    wqT = nc.declare_dram_parameter("wqT", [EMBED, EMBED], bf16, isOutput=False)
    wvT = nc.declare_dram_parameter("wvT", [EMBED, EMBED], bf16, isOutput=False)
    woT = nc.declare_dram_parameter("woT", [EMBED, EMBED], bf16, isOutput=False)
    ctm2 = nc.declare_dram_parameter("ctm2", [EMBED, NSPL], bf16, isOutput=False)
    bq2 = nc.declare_dram_parameter("bq2", [P, KC], f32, isOutput=False)
    bvb = nc.declare_dram_parameter("bvb", [P, EMBED], f32, isOutput=False)
    bob = nc.declare_dram_parameter("bob", [P, EMBED], f32, isOutput=False)
    scn = nc.declare_dram_parameter("scn", [NSPL, 1], f32, isOutput=False)
    bgs = nc.declare_dram_parameter("bgs", [NSPL, 1], f32, isOutput=False)
    one64 = nc.declare_dram_parameter("one64", [P, NSPL], bf16, isOutput=False)
    eyeb = nc.declare_dram_parameter("eyeb", [P, P], bf16, isOutput=False)
    eyef = nc.declare_dram_parameter("eyef", [P, P], f32, isOutput=False)
    oneb = nc.declare_dram_parameter("oneb", [NSPL, P], bf16, isOutput=False)
    y = nc.declare_dram_parameter("y", [SOWN, EMBED], f32, isOutput=True)

    with TileContext(nc) as tc:
        cpool_cm = tc.tile_pool(name="const", bufs=1)
        cpool = cpool_cm.__enter__()
        bq_sb = cpool.tile([P, KC], f32)
        bv_sb = cpool.tile([P, EMBED], f32)
        bo_sb = cpool.tile([P, EMBED], f32)
        sc_sb = cpool.tile([NSPL, 1], f32)
        bg_sb = cpool.tile([NSPL, 1], f32)
        o64_sb = cpool.tile([P, NSPL], bf16)
        eyeb_sb = cpool.tile([P, P], bf16)
        eyef_sb = cpool.tile([P, P], f32)
        oneb_sb = cpool.tile([NSPL, P], bf16)
        ct_sb = cpool.tile([P, KC, NSPL], bf16)
        gt = cpool.tile([NSPL, SOWN], bf16)     # G'^T own: [n, s_own]
        gT = cpool.tile([P, SCH, NSPL], bf16)   # G' own:   [t_own, n]
        gs_own = cpool.tile([NSPL, 2], f32)     # per-half gsum accum

        nc.sync.dma_start(bq_sb[:], bq2[:])
        nc.sync.dma_start(sc_sb[:], scn[:])
        nc.sync.dma_start(bg_sb[:], bgs[:])
        nc.sync.dma_start(o64_sb[:], one64[:])
        nc.sync.dma_start(eyeb_sb[:], eyeb[:])
        nc.sync.dma_start(eyef_sb[:], eyef[:])
        nc.sync.dma_start(oneb_sb[:], oneb[:])
        ctr = ctm2.rearrange("(k p) n -> k p n", p=P)
        for k in range(KC):
            nc.sync.dma_start(ct_sb[:, k], ctr[k])
        nc.sync.dma_start(bv_sb[:], bvb[:])
        nc.sync.dma_start(bo_sb[:], bob[:])

        # ---------------- Phase A: q side (transpose, project, G') ----------
        with tc.tile_pool(name="pa", bufs=1) as pa, \
             tc.tile_pool(name="qe", bufs=3) as qep, \
             tc.tile_pool(name="sqe", bufs=3) as sqp, \
             tc.tile_pool(name="pst", bufs=2, space="PSUM") as pst, \
             tc.tile_pool(name="psq", bufs=4, space="PSUM") as psq, \
             tc.tile_pool(name="psd2", bufs=2, space="PSUM") as psd2:
            wq = pa.tile([P, KC, EMBED], bf16)
            wqr = wqT.rearrange("(k p) e -> k p e", p=P)
            for k in range(KC):
                nc.sync.dma_start(wq[:, k], wqr[k])
            xq_nat = pa.tile([P, SCH, EMBED], f32)
            xqr = xq.rearrange("(c p) d -> c p d", p=P)
            for c in range(SCH):
                nc.sync.dma_start(xq_nat[:, c], xqr[c])
            xqT = pa.tile([P, KC, SOWN], bf16)
            for dch in range(KC):
                for s2 in range(2):
                    tp = pst.tile([P, 512], f32, tag="tp")
                    for k in range(4):
                        sch = s2 * 4 + k
                        nc.tensor.transpose(
                            tp[:, ts(k, P)],
                            xq_nat[:, sch, ts(dch, P)], eyef_sb[:])
                    if (dch + s2) % 2 == 0:
                        nc.scalar.activation(xqT[:, dch, ts(s2, 512)], tp,
                                             AF.Copy)
                    else:
                        nc.vector.tensor_copy(xqT[:, dch, ts(s2, 512)], tp)

            d2ps = [psd2.tile([NSPL, 512], f32, tag="d2", name=f"d2ps{i}")
                    for i in range(2)]
            for e in range(KC):
                qps = [psq.tile([P, 512], f32, tag="qps", name=f"qps{e}_{i}")
                       for i in range(2)]
                for k in range(KC):
                    for s2 in range(2):
                        nc.tensor.matmul(
                            qps[s2], wq[:, k, ts(e, P)],
                            xqT[:, k, ts(s2, 512)],
                            start=(k == 0), stop=(k == KC - 1))
                qe = qep.tile([P, SOWN], bf16, tag="qe")
                for s2 in range(2):
                    if s2 == 0:
                        nc.scalar.activation(qe[:, ts(s2, 512)], qps[s2],
                                             AF.Identity, bias=bq_sb[:, ds(e, 1)])
                    else:
                        nc.vector.tensor_scalar_add(qe[:, ts(s2, 512)], qps[s2],
                                                    bq_sb[:, ds(e, 1)])
                sq = sqp.tile([P, SOWN], bf16, tag="sq")
                nc.vector.tensor_mul(sq, qe, qe)
                for s2 in range(2):
                    nc.tensor.matmul(d2ps[s2], ct_sb[:, e], qe[:, ts(s2, 512)],
                                     start=(e == 0), stop=False)
                    nc.tensor.matmul(d2ps[s2], o64_sb[:], sq[:, ts(s2, 512)],
                                     start=False, stop=(e == KC - 1))
            # G' = exp(-inv2v*d2 + 0.5*ln(amp)); accum_out -> per-half gsum
            for s2 in range(2):
                nc.scalar.activation(gt[:, ts(s2, 512)], d2ps[s2], AF.Exp,
                                     bias=bg_sb[:], scale=sc_sb[:],
                                     accum_out=gs_own[:, ds(s2, 1)])
        # gT = transpose(gt): [t_own, n] chunks
        with tc.tile_pool(name="pstg", bufs=2, space="PSUM") as pstg:
            for tch in range(SCH):
                tp = pstg.tile([P, NSPL], bf16, tag="tpg")
                nc.tensor.transpose(tp[:], gt[:, ts(tch, P)],
                                    eyeb_sb[0:NSPL, 0:NSPL])
                if tch % 2 == 0:
                    nc.vector.tensor_copy(gT[:, tch], tp)
                else:
                    nc.scalar.activation(gT[:, tch], tp, AF.Copy)

        # ---------------- Phase B: v side (transpose, project, M) -----------
        mpool_cm = tc.tile_pool(name="mpool", bufs=1)
        mpool = mpool_cm.__enter__()
        m_sb = mpool.tile([NSPL, MW], f32)
        mr_sb = mpool.tile([NSPL, MW], f32)
        mb = mpool.tile([NSPL, EMBED], bf16)
        gsumb = mpool.tile([NSPL, P], bf16)
        with tc.tile_pool(name="pb", bufs=1) as pb, \
             tc.tile_pool(name="pstv", bufs=2, space="PSUM") as pstv, \
             tc.tile_pool(name="psv", bufs=4, space="PSUM") as psv, \
             tc.tile_pool(name="psm", bufs=2, space="PSUM") as psm, \
             tc.tile_pool(name="dram", bufs=1, space="DRAM") as dram:
            wv = pb.tile([P, KC, EMBED], bf16)
            wvr = wvT.rearrange("(k p) e -> k p e", p=P)
            for k in range(KC):
                nc.sync.dma_start(wv[:, k], wvr[k])
            xv_nat = pb.tile([P, SCH, EMBED], f32)
            xvr = xv.rearrange("(c p) d -> c p d", p=P)
            for c in range(SCH):
                nc.sync.dma_start(xv_nat[:, c], xvr[c])
            xvT = pb.tile([P, KC, SOWN], bf16)
            for dch in range(KC):
                for s2 in range(2):
                    tp = pstv.tile([P, 512], f32, tag="tpv")
                    for k in range(4):
                        sch = s2 * 4 + k
                        nc.tensor.transpose(
                            tp[:, ts(k, P)],
                            xv_nat[:, sch, ts(dch, P)], eyef_sb[:])
                    if (dch + s2) % 2 == 0:
                        nc.scalar.activation(xvT[:, dch, ts(s2, 512)], tp,
                                             AF.Copy)
                    else:
                        nc.vector.tensor_copy(xvT[:, dch, ts(s2, 512)], tp)

            v_sb = pb.tile([P, SCH, EMBED], bf16)
            mps = [psm.tile([NSPL, 512], f32, tag="mps", name=f"mps{i}")
                   for i in range(2)]
            for t in range(SCH):
                vps = [psv.tile([P, 512], f32, tag="vps", name=f"vps{t}_{i}")
                       for i in range(2)]
                for k in range(KC):
                    for eh in range(2):
                        nc.tensor.matmul(
                            vps[eh], xvT[:, k, ts(t, P)],
                            wv[:, k, ts(eh, 512)],
                            start=(k == 0), stop=(k == KC - 1))
                for eh in range(2):
                    nc.vector.tensor_add(v_sb[:, t, ts(eh, 512)], vps[eh],
                                         bv_sb[:, ts(eh, 512)])
                for mh in range(2):
                    nc.tensor.matmul(mps[mh], gT[:, t],
                                     v_sb[:, t, ts(mh, 512)],
                                     start=(t == 0), stop=(t == SCH - 1))
            # pack M halves + per-half gsum, pair AllReduce
            nc.scalar.activation(m_sb[:, 0:512], mps[0], AF.Copy)
            nc.vector.tensor_copy(m_sb[:, 512:1024], mps[1])
            nc.vector.tensor_copy(m_sb[:, EMBED:MW], gs_own)
            md_in = dram.tile([NSPL, MW], f32)
            md_out = dram.tile([NSPL, MW], f32)
            nc.sync.dma_start(md_in[:], m_sb[:])
            nc.gpsimd.collective_compute(
                "AllReduce", mybir.AluOpType.add,
                replica_groups=[[0, 1], [2, 3], [4, 5], [6, 7]],
                ins=[md_in[:].opt()], outs=[md_out[:].opt()])
            nc.sync.dma_start(mr_sb[:], md_out[:])
            nc.vector.tensor_copy(mb[:], mr_sb[:, 0:EMBED])
            # gsum = sum of the 4 reduced half-sums; broadcast to [64,128]
            gsum = mpool.tile([NSPL, 1], f32)
            nc.vector.tensor_add(gsum, mr_sb[:, ds(EMBED, 1)],
                                 mr_sb[:, ds(EMBED + 1, 1)])
            nc.vector.tensor_scalar_mul(gsumb, oneb_sb[:], gsum)

        # ---------------- Phase C: rs, outT ----------------
        otpool_cm = tc.tile_pool(name="otpool", bufs=1)
        otpool = otpool_cm.__enter__()
        outT = otpool.tile([P, KC, SOWN], bf16)
        wo = otpool.tile([P, KC, EMBED], bf16)
        wor = woT.rearrange("(k p) e -> k p e", p=P)
        for k in range(KC):
            nc.sync.dma_start(wo[:, k], wor[k])
        with tc.tile_pool(name="rsp", bufs=1) as rsp, \
             tc.tile_pool(name="psrs", bufs=2, space="PSUM") as psrs, \
             tc.tile_pool(name="pso", bufs=4, space="PSUM") as pso:
            rsin = rsp.tile([P, SOWN], f32)
            rs_tmp = rsp.tile([P, SOWN], f32)
            for s2 in range(2):
                rps = psrs.tile([P, 512], f32, tag="rps")
                nc.tensor.matmul(rps, gsumb, gt[:, ts(s2, 512)],
                                 start=True, stop=True)
                nc.vector.tensor_scalar_add(rs_tmp[:, ts(s2, 512)], rps, EPS)
                nc.vector.reciprocal(rsin[:, ts(s2, 512)],
                                     rs_tmp[:, ts(s2, 512)])
            for dch in range(KC):
                for s2 in range(2):
                    ops = pso.tile([P, 512], f32, tag="ops")
                    nc.tensor.matmul(ops, mb[:, ts(dch, P)],
                                     gt[:, ts(s2, 512)],
                                     start=True, stop=True)
                    nc.vector.tensor_mul(outT[:, dch, ts(s2, 512)], ops,
                                         rsin[:, ts(s2, 512)])

        # ---------------- Phase D: output projection ----------------
        with tc.tile_pool(name="ybuf", bufs=2) as yb, \
             tc.tile_pool(name="psy", bufs=4, space="PSUM") as psy:
            yr = y.rearrange("(c p) e -> c p e", p=P)
            for sc in range(SCH):
                yps = psy.tile([P, EMBED], f32, tag="yps")
                for k in range(KC):
                    for eh in range(2):
                        nc.tensor.matmul(
                            yps[:, ts(eh, 512)], outT[:, k, ts(sc, P)],
                            wo[:, k, ts(eh, 512)],
                            start=(k == 0), stop=(k == KC - 1))
                ysb = yb.tile([P, EMBED], f32, tag="ysb")
                nc.vector.tensor_add(ysb, yps, bo_sb)
                nc.sync.dma_start(yr[sc], ysb)
        otpool_cm.__exit__(None, None, None)
        mpool_cm.__exit__(None, None, None)
        cpool_cm.__exit__(None, None, None)

    nc.finalize()
    return nc


def _const_arrays(Wq, bq, Wv, bv, Wo, bo, C, ls, amp):
    """Host-side constant prep (cached; runs once per weight set)."""
    f = np.float32
    Wq = np.asarray(Wq, f); bq = np.asarray(bq, f)
    Wv = np.asarray(Wv, f); bv = np.asarray(bv, f)
    Wo = np.asarray(Wo, f); bo = np.asarray(bo, f)
    C = np.asarray(C, f); ls = np.asarray(ls, f); amp = np.asarray(amp, f)
    inv2v = 0.5 * np.exp(-2.0 * ls).astype(f)
    c2 = (C.astype(np.float64) ** 2).sum(1)
    out = {
        "wqT": np.ascontiguousarray(Wq.T).astype(BF16),
        "wvT": np.ascontiguousarray(Wv.T).astype(BF16),
        "woT": np.ascontiguousarray(Wo.T).astype(BF16),
        "ctm2": np.ascontiguousarray((-2.0 * C).T).astype(BF16),
        "bq2": np.ascontiguousarray(bq.reshape(KC, P).T),
        "bvb": np.ascontiguousarray(np.broadcast_to(bv, (P, EMBED))),
        "bob": np.ascontiguousarray(np.broadcast_to(bo, (P, EMBED))),
        "scn": (-inv2v).reshape(NSPL, 1).astype(f),
        # fold sqrt(amp) into G': exp(x + 0.5 ln amp)
        "bgs": (-inv2v * c2 + 0.5 * np.log(np.maximum(amp, 1e-38))
                ).reshape(NSPL, 1).astype(f),
        "one64": np.ones((P, NSPL), BF16),
        "eyeb": np.eye(P, dtype=BF16),
        "eyef": np.eye(P, dtype=f),
        "oneb": np.ones((NSPL, P), BF16),
    }
    return out


def _weights_key(arrs):
    """Cheap content fingerprint: data pointer + shape + sampled bytes."""
    import hashlib
    h = hashlib.blake2b(digest_size=16)
    for a in arrs:
        a = np.asarray(a)
        ai = a.__array_interface__
        h.update(str((ai["data"][0], a.shape, str(a.dtype))).encode())
        raw = a.reshape(-1)
        step = max(1, raw.size // 4096)
        h.update(np.ascontiguousarray(raw[::step]).tobytes())
    return h.digest()


def _get_dispatch():
    """Build program + jit once; returns dispatch closure state."""
    global _PROG, _DISPATCH
    if _DISPATCH is not None:
        return _DISPATCH
    import jax
    import jax.numpy as jnp
    from jax.sharding import Mesh, PartitionSpec, NamedSharding
    from jax.experimental.shard_map import shard_map
    import concourse.mybir as mybir
    from concourse.bass2jax import (_bass_exec_p, partition_id_tensor,
                                    install_neuronx_cc_hook)

    if _PROG is None:
        _PROG = _build_program()
    nc = _PROG
    install_neuronx_cc_hook()

    in_names = []
    out_names = []
    out_avals = []
    for alloc in nc.m.functions[0].allocations:
        if not isinstance(alloc, mybir.MemoryLocationSet):
            continue
        name = alloc.memorylocations[0].name
        if alloc.kind == "ExternalInput":
            if nc.partition_id_tensor is None or name != nc.partition_id_tensor.name:
                in_names.append(name)
        elif alloc.kind == "ExternalOutput":
            out_names.append(name)
            out_avals.append(jax.core.ShapedArray(
                tuple(alloc.tensor_shape), mybir.dt.np(alloc.dtype)))
    n_params = len(in_names)
    all_names = in_names + out_names
    if nc.partition_id_tensor is not None:
        all_names.append(nc.partition_id_tensor.name)

    def _body(*args):
        operands = list(args)
        if nc.partition_id_tensor is not None:
            operands.append(partition_id_tensor())
        outs = _bass_exec_p.bind(
            *operands, out_avals=tuple(out_avals), in_names=tuple(all_names),
            out_names=tuple(out_names), lowering_input_output_aliases=(),
            sim_require_finite=True, sim_require_nnan=True, nc=nc)
        return tuple(outs)

    devices = jax.devices()[:NCORES]
    mesh = Mesh(np.asarray(devices), ("core",))
    n_outs = len(out_names)
    sharded = jax.jit(
        shard_map(_body, mesh=mesh,
                  in_specs=(PartitionSpec("core"),) * (n_params + n_outs),
                  out_specs=(PartitionSpec("core"),) * n_outs,
                  check_rep=False),
        donate_argnums=tuple(range(n_params, n_params + n_outs)))
    zeros_fn = jax.jit(
        lambda: jnp.zeros((NCORES * SOWN, EMBED), jnp.float32),
        out_shardings=NamedSharding(mesh, PartitionSpec("core")))

    _DISPATCH = dict(in_names=in_names, mesh=mesh, sharded=sharded,
                     zeros_fn=zeros_fn, jax=jax)
    return _DISPATCH


def _get_consts(disp, Wq, bq, Wv, bv, Wo, bo, C, ls, amp):
    """Device-resident constant arrays, cached across calls by content key."""
    global _CONSTS
    ws = (Wq, bq, Wv, bv, Wo, bo, C, ls, amp)
    key = _weights_key(ws)
    if _CONSTS is not None and _CONSTS[0] == key:
        return _CONSTS[1]
    import jax
    from jax.sharding import NamedSharding, PartitionSpec
    host = _const_arrays(*ws)
    dev = {}
    for name, arr in host.items():
        # replicate: stack per-core copies along axis 0 (in_spec P("core"))
        stacked = np.broadcast_to(
            arr, (NCORES,) + arr.shape).reshape(NCORES * arr.shape[0],
                                                *arr.shape[1:])
        dev[name] = jax.device_put(
            np.ascontiguousarray(stacked),
            NamedSharding(disp["mesh"], PartitionSpec("core")))
    _CONSTS = (key, dev)
    return dev


def run_cores(inputs, trace=False):
    """Run the SPMD kernel; returns (full_output, None)."""
    disp = _get_dispatch()
    q = np.asarray(inputs["query"], np.float32)
    v = np.asarray(inputs["value"], np.float32)
    consts = _get_consts(
        disp, inputs["Wq"], inputs["bq"], inputs["Wv"], inputs["bv"],
        inputs["Wo"], inputs["bo"], inputs["splat_centers"],
        inputs["splat_log_scales"], inputs["splat_amplitudes"])
    q8 = np.ascontiguousarray(q).reshape(NCORES * SOWN, EMBED)
    v8 = np.ascontiguousarray(v).reshape(NCORES * SOWN, EMBED)
    args = []
    for name in disp["in_names"]:
        if name == "xq":
            args.append(q8)
        elif name == "xv":
            args.append(v8)
        else:
            args.append(consts[name])
    zeros = disp["zeros_fn"]()
    out_arrs = disp["sharded"](*args, zeros)
    out = np.asarray(out_arrs[0]).reshape(B, S, EMBED)
    return out, None


def kernel(**inputs):
    out, _ = run_cores(inputs, trace=False)
    return out


# revision 11
# speedup vs baseline: 2.0972x; 1.6364x over previous
"""HSA (hierarchical splat attention) Bass kernel for Trainium2, 8 NeuronCores.

Math (per batch b):
    q = query @ Wq.T + bq                      [S, D]
    v = value @ Wv.T + bv                      [S, D]
    d2[s,n]  = |q_s|^2 - 2 q_s.c_n + |c_n|^2
    G[s,n]   = exp(-d2[s,n] * inv2v[n]),  inv2v = 0.5*exp(-2*log_scales)
    Asym[s,t]= sum_n G[s,n]*amp[n]*G[t,n]      (rank-N_SPLATS!)
    A        = Asym / (rowsum(Asym) + eps)
    out      = A @ v ;  y = out @ Wo.T + bo

Everything downstream of G is pushed through the rank-64 bottleneck
(G' = G*sqrt(amp), Asym = G'G'^T is never materialized):
    P    = G'^T @ Xv                [N, D]   (raw values - no v-projection!)
    M    = P @ Wv.T + gsum x bv     [N, D]
    W2   = M @ Wo.T                 [N, D]
    y[s] = (G'[s,:] @ W2) / (G'[s,:].gsum + eps) + bo
where gsum = G'^T @ 1.  The only full-size GEMM left is the q-projection
(needed for |q_s|^2 inside d2).

Sharding: core c = (batch b = c//2, seq-half h = c%2). Each core receives its
own 1024-token halves of query/value as contiguous f32 views (zero host prep),
PE-transposes Xq on device, and a single pair-wise AllGather of [64,1026] f32
(P_own + per-half gsum) completes the token contraction. Weights/constants are
content-hash cached device-resident arrays, so steady-state host->device
traffic is just the q,v halves in and y out.
"""

import numpy as np
import ml_dtypes

BF16 = ml_dtypes.bfloat16
EMBED = 1024
S = 2048
NSPL = 64
B = 4
NCORES = 8
P = 128
KC = EMBED // P   # 8 chunks over d/e
SOWN = S // 2     # 1024 own tokens per core
SCH = SOWN // P   # 8 own s/t chunks
MW = EMBED + 2    # AllGather payload: 1024 P-cols + 2 gsum half-cols
EPS = 1e-8

_PROG = None       # cached bass program
_DISPATCH = None   # cached jit etc.
_CONSTS = None     # cached (key, device_arrays)


def _build_program():
    import concourse.mybir as mybir
    from concourse import bacc
    from concourse.tile import TileContext
    from concourse.bass import ts, ds

    f32 = mybir.dt.float32
    bf16 = mybir.dt.bfloat16
    AF = mybir.ActivationFunctionType

    nc = bacc.Bacc("TRN2", target_bir_lowering=False, debug=False,
                   num_devices=NCORES)
    xq = nc.declare_dram_parameter("xq", [SOWN, EMBED], f32, isOutput=False)
    xv = nc.declare_dram_parameter("xv", [SOWN, EMBED], f32, isOutput=False)
    wqT = nc.declare_dram_parameter("wqT", [EMBED, EMBED], bf16, isOutput=False)
    wvT = nc.declare_dram_parameter("wvT", [EMBED, EMBED], bf16, isOutput=False)
    woT = nc.declare_dram_parameter("woT", [EMBED, EMBED], bf16, isOutput=False)
    ctm2 = nc.declare_dram_parameter("ctm2", [EMBED, NSPL], bf16, isOutput=False)
    bq2 = nc.declare_dram_parameter("bq2", [P, KC], f32, isOutput=False)
    bvb = nc.declare_dram_parameter("bvb", [P, EMBED], f32, isOutput=False)
    bob = nc.declare_dram_parameter("bob", [P, EMBED], f32, isOutput=False)
    scn = nc.declare_dram_parameter("scn", [NSPL, 1], f32, isOutput=False)
    bgs = nc.declare_dram_parameter("bgs", [NSPL, 1], f32, isOutput=False)
    one64 = nc.declare_dram_parameter("one64", [P, NSPL], bf16, isOutput=False)
    eyeb = nc.declare_dram_parameter("eyeb", [P, P], bf16, isOutput=False)
    eyef = nc.declare_dram_parameter("eyef", [P, P], f32, isOutput=False)
    y = nc.declare_dram_parameter("y", [SOWN, EMBED], f32, isOutput=True)

    with TileContext(nc) as tc:
        cpool_cm = tc.tile_pool(name="const", bufs=1)
        cpool = cpool_cm.__enter__()
        bq_sb = cpool.tile([P, KC], f32)
        bv_sb = cpool.tile([P, EMBED], f32)
        bo_sb = cpool.tile([P, EMBED], f32)
        sc_sb = cpool.tile([NSPL, 1], f32)
        bg_sb = cpool.tile([NSPL, 1], f32)
        o64_sb = cpool.tile([P, NSPL], bf16)
        eyeb_sb = cpool.tile([P, P], bf16)
        eyef_sb = cpool.tile([P, P], f32)
        ct_sb = cpool.tile([P, KC, NSPL], bf16)
        gt = cpool.tile([NSPL, SOWN], bf16)     # G'^T own: [n, s_own]
        gT = cpool.tile([P, SCH, NSPL], bf16)   # G' own:   [t_own, n]
        gs_own = cpool.tile([NSPL, 2], f32)     # per-half gsum accum
        xvb = cpool.tile([P, SCH, EMBED], bf16)  # Xv own, natural, bf16

        nc.sync.dma_start(bq_sb[:], bq2[:])
        nc.sync.dma_start(sc_sb[:], scn[:])
        nc.sync.dma_start(bg_sb[:], bgs[:])
        nc.scalar.dma_start(o64_sb[:], one64[:])
        nc.scalar.dma_start(eyeb_sb[:], eyeb[:])
        nc.sync.dma_start(eyef_sb[:], eyef[:])
        ctr = ctm2.rearrange("(h c p) n -> h p c n", p=P, c=4)
        for k in range(2):
            nc.scalar.dma_start(ct_sb[:, k * 4:(k + 1) * 4], ctr[k])
        nc.scalar.dma_start(bv_sb[:], bvb[:])
        nc.sync.dma_start(bo_sb[:], bob[:])

        # ---------------- Phase A: q side (load, transpose, project, G') ----
        with tc.tile_pool(name="pa", bufs=1) as pa, \
             tc.tile_pool(name="qe", bufs=3) as qep, \
             tc.tile_pool(name="sqe", bufs=3) as sqp, \
             tc.tile_pool(name="pst", bufs=2, space="PSUM") as pst, \
             tc.tile_pool(name="psq", bufs=4, space="PSUM") as psq, \
             tc.tile_pool(name="psd2", bufs=2, space="PSUM") as psd2:
            wq = pa.tile([P, KC, EMBED], bf16)
            wqr = wqT.rearrange("(h c p) e -> h p c e", p=P, c=4)
            for k in range(2):
                nc.sync.dma_start(wq[:, k * 4:(k + 1) * 4], wqr[k])
            xq_nat = pa.tile([P, SCH, EMBED], f32)
            xqr = xq.rearrange("(g c p) d -> g p c d", p=P, c=2)
            for g in range(4):
                eng = nc.sync if g % 2 == 0 else nc.scalar
                eng.dma_start(xq_nat[:, g * 2:(g + 1) * 2], xqr[g])
            # load + cast Xv early too (needed right after G')
            xv_nat = pa.tile([P, SCH, EMBED], f32)
            xvr = xv.rearrange("(g c p) d -> g p c d", p=P, c=2)
            for g in range(4):
                eng = nc.scalar if g % 2 == 0 else nc.sync
                eng.dma_start(xv_nat[:, g * 2:(g + 1) * 2], xvr[g])
            for c in range(SCH):
                nc.gpsimd.tensor_copy(xvb[:, c], xv_nat[:, c])

            # bf16 cast of Xq then PE-transpose 128x128 tiles
            xqb = pa.tile([P, SCH, EMBED], bf16)
            for c in range(SCH):
                nc.vector.tensor_copy(xqb[:, c], xq_nat[:, c])
            xqT = pa.tile([P, KC, SOWN], bf16)
            for dch in range(KC):
                for s2 in range(2):
                    tp = pst.tile([P, 512], bf16, tag="tp")
                    for k in range(4):
                        sch = s2 * 4 + k
                        nc.tensor.transpose(
                            tp[:, ts(k, P)],
                            xqb[:, sch, ts(dch, P)], eyeb_sb[:])
                    if (dch + s2) % 2 == 0:
                        nc.scalar.activation(xqT[:, dch, ts(s2, 512)], tp,
                                             AF.Copy)
                    else:
                        nc.vector.tensor_copy(xqT[:, dch, ts(s2, 512)], tp)

            d2ps = [psd2.tile([NSPL, 512], f32, tag="d2", name=f"d2ps{i}")
                    for i in range(2)]
            for e in range(KC):
                qps = [psq.tile([P, 512], f32, tag="qps", name=f"qps{e}_{i}")
                       for i in range(2)]
                for k in range(KC):
                    for s2 in range(2):
                        nc.tensor.matmul(
                            qps[s2], wq[:, k, ts(e, P)],
                            xqT[:, k, ts(s2, 512)],
                            start=(k == 0), stop=(k == KC - 1))
                qe = qep.tile([P, SOWN], bf16, tag="qe")
                for s2 in range(2):
                    if s2 == 0:
                        nc.scalar.activation(qe[:, ts(s2, 512)], qps[s2],
                                             AF.Identity, bias=bq_sb[:, ds(e, 1)])
                    else:
                        nc.vector.tensor_scalar_add(qe[:, ts(s2, 512)], qps[s2],
                                                    bq_sb[:, ds(e, 1)])
                sq = sqp.tile([P, SOWN], bf16, tag="sq")
                nc.vector.tensor_mul(sq, qe, qe)
                for s2 in range(2):
                    nc.tensor.matmul(d2ps[s2], ct_sb[:, e], qe[:, ts(s2, 512)],
                                     start=(e == 0), stop=False)
                    nc.tensor.matmul(d2ps[s2], o64_sb[:], sq[:, ts(s2, 512)],
                                     start=False, stop=(e == KC - 1))
            # G' = exp(-inv2v*d2 + (-inv2v*c2 + 0.5*ln amp)); accum -> gsum
            for s2 in range(2):
                nc.scalar.activation(gt[:, ts(s2, 512)], d2ps[s2], AF.Exp,
                                     bias=bg_sb[:], scale=sc_sb[:],
                                     accum_out=gs_own[:, ds(s2, 1)])

        # gT = transpose(gt): [t_own, n] chunks
        with tc.tile_pool(name="pstg", bufs=2, space="PSUM") as pstg:
            for tch in range(SCH):
                tp = pstg.tile([P, NSPL], bf16, tag="tpg")
                nc.tensor.transpose(tp[:], gt[:, ts(tch, P)],
                                    eyeb_sb[0:NSPL, 0:NSPL])
                if tch % 2 == 0:
                    nc.vector.tensor_copy(gT[:, tch], tp)
                else:
                    nc.scalar.activation(gT[:, tch], tp, AF.Copy)

        # ---------------- Phase B: P = G'^T Xv, pair AllGather ----------
        mpool_cm = tc.tile_pool(name="mpool", bufs=1)
        mpool = mpool_cm.__enter__()
        m_sb = mpool.tile([NSPL, MW], f32)
        pr_sb = mpool.tile([NSPL, 2, MW], f32)
        with tc.tile_pool(name="psP", bufs=2, space="PSUM") as psP, \
             tc.tile_pool(name="dram", bufs=1, space="DRAM") as dram:
            pps = [psP.tile([NSPL, 512], f32, tag="pps", name=f"pps{i}")
                   for i in range(2)]
            for t in range(SCH):
                for mh in range(2):
                    nc.tensor.matmul(pps[mh], gT[:, t],
                                     xvb[:, t, ts(mh, 512)],
                                     start=(t == 0), stop=(t == SCH - 1))
            nc.scalar.activation(m_sb[:, 0:512], pps[0], AF.Copy)
            nc.vector.tensor_copy(m_sb[:, 512:1024], pps[1])
            nc.vector.tensor_copy(m_sb[:, EMBED:MW], gs_own)
            md_in = dram.tile([NSPL, MW], f32)
            md_out = dram.tile([2, NSPL, MW], f32)
            nc.sync.dma_start(md_in[:], m_sb[:])
            nc.gpsimd.collective_compute(
                "AllGather", mybir.AluOpType.bypass,
                replica_groups=[[0, 1], [2, 3], [4, 5], [6, 7]],
                ins=[md_in[:].opt()], outs=[md_out[:].opt()])
            mdv = md_out.rearrange("h n w -> n h w")
            nc.sync.dma_start(pr_sb[:], mdv)

        # ---------------- Phase C: M, W2, rs (all rank-64) ----------
        wpool_cm = tc.tile_pool(name="wpool", bufs=1)
        wpool = wpool_cm.__enter__()
        wv = wpool.tile([P, KC, EMBED], bf16)
        wo = wpool.tile([P, KC, EMBED], bf16)
        wvr = wvT.rearrange("(h c p) e -> h p c e", p=P, c=4)
        wor = woT.rearrange("(h c p) e -> h p c e", p=P, c=4)
        for k in range(2):
            nc.sync.dma_start(wv[:, k * 4:(k + 1) * 4], wvr[k])
            nc.scalar.dma_start(wo[:, k * 4:(k + 1) * 4], wor[k])

        w2 = wpool.tile([NSPL, EMBED], bf16)
        rsin = wpool.tile([P, SCH], f32)
        gsc = wpool.tile([NSPL, 1], bf16)
        with tc.tile_pool(name="pc", bufs=1) as pc, \
             tc.tile_pool(name="psPT", bufs=2, space="PSUM") as psPT, \
             tc.tile_pool(name="psmT", bufs=2, space="PSUM") as psmT, \
             tc.tile_pool(name="psM", bufs=2, space="PSUM") as psM, \
             tc.tile_pool(name="psW", bufs=2, space="PSUM") as psW:
            p_red = pc.tile([NSPL, EMBED], f32)
            nc.vector.tensor_add(p_red, pr_sb[:, 0, 0:EMBED],
                                 pr_sb[:, 1, 0:EMBED])
            gs2 = pc.tile([NSPL, 2], f32)
            nc.vector.tensor_add(gs2, pr_sb[:, 0, EMBED:MW],
                                 pr_sb[:, 1, EMBED:MW])
            gsum = pc.tile([NSPL, 1], f32)
            nc.vector.tensor_add(gsum, gs2[:, 0:1], gs2[:, 1:2])
            nc.vector.tensor_copy(gsc, gsum)
            # PT = transpose(P_red) -> [e, n] bf16
            pT = pc.tile([P, KC, NSPL], bf16)
            for ech in range(KC):
                tp = psPT.tile([P, NSPL], f32, tag="tpt")
                nc.tensor.transpose(tp[:], p_red[:, ts(ech, P)],
                                    eyef_sb[0:NSPL, 0:NSPL])
                if ech % 2 == 0:
                    nc.vector.tensor_copy(pT[:, ech], tp)
                else:
                    nc.scalar.activation(pT[:, ech], tp, AF.Copy)
            # M = PT.T @ WvT  (+ gsum x bv added during psum->sbuf)
            mps = [psM.tile([NSPL, 512], f32, tag="mps", name=f"mps{i}")
                   for i in range(2)]
            for ech in range(KC):
                for mh in range(2):
                    nc.tensor.matmul(mps[mh], pT[:, ech],
                                     wv[:, ech, ts(mh, 512)],
                                     start=(ech == 0), stop=(ech == KC - 1))
            gbv = pc.tile([NSPL, EMBED], f32)
            nc.vector.tensor_scalar_mul(gbv, bv_sb[0:NSPL, :], gsum)
            mb = pc.tile([NSPL, EMBED], bf16)
            for mh in range(2):
                nc.vector.tensor_add(mb[:, ts(mh, 512)], mps[mh],
                                     gbv[:, ts(mh, 512)])
            # mT = transpose(M) -> [d, n] bf16
            mT = pc.tile([P, KC, NSPL], bf16)
            for dch in range(KC):
                tp = psmT.tile([P, NSPL], bf16, tag="tpm")
                nc.tensor.transpose(tp[:], mb[:, ts(dch, P)],
                                    eyeb_sb[0:NSPL, 0:NSPL])
                if dch % 2 == 0:
                    nc.vector.tensor_copy(mT[:, dch], tp)
                else:
                    nc.scalar.activation(mT[:, dch], tp, AF.Copy)
            # W2 = mT.T @ WoT
            wps = [psW.tile([NSPL, 512], f32, tag="wps", name=f"wps{i}")
                   for i in range(2)]
            for dch in range(KC):
                for eh in range(2):
                    nc.tensor.matmul(wps[eh], mT[:, dch],
                                     wo[:, dch, ts(eh, 512)],
                                     start=(dch == 0), stop=(dch == KC - 1))
            for eh in range(2):
                if eh == 0:
                    nc.scalar.activation(w2[:, ts(eh, 512)], wps[eh], AF.Copy)
                else:
                    nc.vector.tensor_copy(w2[:, ts(eh, 512)], wps[eh])
        # ---------------- Phase D: rs, then y = (G' @ W2) * rsin + bo ----------
        with tc.tile_pool(name="ybuf", bufs=3) as yb, \
             tc.tile_pool(name="psrs", bufs=1, space="PSUM") as psrs, \
             tc.tile_pool(name="psy", bufs=3, space="PSUM") as psy:
            rsc = psrs.tile([P, SCH], f32, tag="rsc")
            for sch in range(SCH):
                nc.tensor.matmul(rsc[:, ds(sch, 1)], gt[:, ts(sch, P)],
                                 gsc, start=True, stop=True)
            rst = yb.tile([P, SCH], f32, tag="rst")
            nc.vector.tensor_scalar_add(rst, rsc, EPS)
            nc.vector.reciprocal(rsin, rst)
            yr = y.rearrange("(c p) e -> c p e", p=P)
            for sc in range(SCH):
                yps = psy.tile([P, EMBED], f32, tag="yps")
                for eh in range(2):
                    nc.tensor.matmul(yps[:, ts(eh, 512)], gt[:, ts(sc, P)],
                                     w2[:, ts(eh, 512)], start=True, stop=True)
                yt = yb.tile([P, EMBED], f32, tag="yt")
                nc.scalar.activation(yt, yps, AF.Identity,
                                     scale=rsin[:, ds(sc, 1)])
                ysb = yb.tile([P, EMBED], f32, tag="ysb")
                nc.vector.tensor_add(ysb, yt, bo_sb)
                eng = nc.sync if sc % 2 == 0 else nc.scalar
                eng.dma_start(yr[sc], ysb)
        wpool_cm.__exit__(None, None, None)
        mpool_cm.__exit__(None, None, None)
        cpool_cm.__exit__(None, None, None)

    nc.finalize()
    return nc


def _const_arrays(Wq, bq, Wv, bv, Wo, bo, C, ls, amp):
    """Host-side constant prep (cached; runs once per weight set)."""
    f = np.float32
    Wq = np.asarray(Wq, f); bq = np.asarray(bq, f)
    Wv = np.asarray(Wv, f); bv = np.asarray(bv, f)
    Wo = np.asarray(Wo, f); bo = np.asarray(bo, f)
    C = np.asarray(C, f); ls = np.asarray(ls, f); amp = np.asarray(amp, f)
    inv2v = 0.5 * np.exp(-2.0 * ls).astype(f)
    c2 = (C.astype(np.float64) ** 2).sum(1)
    out = {
        "wqT": np.ascontiguousarray(Wq.T).astype(BF16),
        "wvT": np.ascontiguousarray(Wv.T).astype(BF16),
        "woT": np.ascontiguousarray(Wo.T).astype(BF16),
        "ctm2": np.ascontiguousarray((-2.0 * C).T).astype(BF16),
        "bq2": np.ascontiguousarray(bq.reshape(KC, P).T),
        "bvb": np.ascontiguousarray(np.broadcast_to(bv, (P, EMBED))),
        "bob": np.ascontiguousarray(np.broadcast_to(bo, (P, EMBED))),
        "scn": (-inv2v).reshape(NSPL, 1).astype(f),
        # fold sqrt(amp) into G': exp(x + 0.5 ln amp)
        "bgs": (-inv2v * c2 + 0.5 * np.log(np.maximum(amp, 1e-38))
                ).reshape(NSPL, 1).astype(f),
        "one64": np.ones((P, NSPL), BF16),
        "eyeb": np.eye(P, dtype=BF16),
        "eyef": np.eye(P, dtype=np.float32),
    }
    return out


def _weights_key(arrs):
    """Cheap content fingerprint: data pointer + shape + sampled bytes."""
    import hashlib
    h = hashlib.blake2b(digest_size=16)
    for a in arrs:
        a = np.asarray(a)
        ai = a.__array_interface__
        h.update(str((ai["data"][0], a.shape, str(a.dtype))).encode())
        raw = a.reshape(-1)
        step = max(1, raw.size // 4096)
        h.update(np.ascontiguousarray(raw[::step]).tobytes())
    return h.digest()


def _get_dispatch():
    """Build program + jit once; returns dispatch closure state."""
    global _PROG, _DISPATCH
    if _DISPATCH is not None:
        return _DISPATCH
    import jax
    import jax.numpy as jnp
    from jax.sharding import Mesh, PartitionSpec, NamedSharding
    from jax.experimental.shard_map import shard_map
    import concourse.mybir as mybir
    from concourse.bass2jax import (_bass_exec_p, partition_id_tensor,
                                    install_neuronx_cc_hook)

    if _PROG is None:
        _PROG = _build_program()
    nc = _PROG
    install_neuronx_cc_hook()

    in_names = []
    out_names = []
    out_avals = []
    for alloc in nc.m.functions[0].allocations:
        if not isinstance(alloc, mybir.MemoryLocationSet):
            continue
        name = alloc.memorylocations[0].name
        if alloc.kind == "ExternalInput":
            if nc.partition_id_tensor is None or name != nc.partition_id_tensor.name:
                in_names.append(name)
        elif alloc.kind == "ExternalOutput":
            out_names.append(name)
            out_avals.append(jax.core.ShapedArray(
                tuple(alloc.tensor_shape), mybir.dt.np(alloc.dtype)))
    n_params = len(in_names)
    all_names = in_names + out_names
    if nc.partition_id_tensor is not None:
        all_names.append(nc.partition_id_tensor.name)

    def _body(*args):
        operands = list(args)
        if nc.partition_id_tensor is not None:
            operands.append(partition_id_tensor())
        outs = _bass_exec_p.bind(
            *operands, out_avals=tuple(out_avals), in_names=tuple(all_names),
            out_names=tuple(out_names), lowering_input_output_aliases=(),
            sim_require_finite=True, sim_require_nnan=True, nc=nc)
        return tuple(outs)

    devices = jax.devices()[:NCORES]
    mesh = Mesh(np.asarray(devices), ("core",))
    n_outs = len(out_names)
    sharded = jax.jit(
        shard_map(_body, mesh=mesh,
                  in_specs=(PartitionSpec("core"),) * (n_params + n_outs),
                  out_specs=(PartitionSpec("core"),) * n_outs,
                  check_rep=False),
        donate_argnums=tuple(range(n_params, n_params + n_outs)))
    zeros_fn = jax.jit(
        lambda: jnp.zeros((NCORES * SOWN, EMBED), jnp.float32),
        out_shardings=NamedSharding(mesh, PartitionSpec("core")))

    _DISPATCH = dict(in_names=in_names, mesh=mesh, sharded=sharded,
                     zeros_fn=zeros_fn, jax=jax)
    return _DISPATCH


def _get_consts(disp, Wq, bq, Wv, bv, Wo, bo, C, ls, amp):
    """Device-resident constant arrays, cached across calls by content key."""
    global _CONSTS
    ws = (Wq, bq, Wv, bv, Wo, bo, C, ls, amp)
    key = _weights_key(ws)
    if _CONSTS is not None and _CONSTS[0] == key:
        return _CONSTS[1]
    import jax
    from jax.sharding import NamedSharding, PartitionSpec
    host = _const_arrays(*ws)
    dev = {}
    for name, arr in host.items():
        # replicate: stack per-core copies along axis 0 (in_spec P("core"))
        stacked = np.broadcast_to(
            arr, (NCORES,) + arr.shape).reshape(NCORES * arr.shape[0],
                                                *arr.shape[1:])
        dev[name] = jax.device_put(
            np.ascontiguousarray(stacked),
            NamedSharding(disp["mesh"], PartitionSpec("core")))
    _CONSTS = (key, dev)
    return dev


def run_cores(inputs, trace=False):
    """Run the SPMD kernel; returns (full_output, None)."""
    disp = _get_dispatch()
    q = np.asarray(inputs["query"], np.float32)
    v = np.asarray(inputs["value"], np.float32)
    consts = _get_consts(
        disp, inputs["Wq"], inputs["bq"], inputs["Wv"], inputs["bv"],
        inputs["Wo"], inputs["bo"], inputs["splat_centers"],
        inputs["splat_log_scales"], inputs["splat_amplitudes"])
    q8 = np.ascontiguousarray(q).reshape(NCORES * SOWN, EMBED)
    v8 = np.ascontiguousarray(v).reshape(NCORES * SOWN, EMBED)
    args = []
    for name in disp["in_names"]:
        if name == "xq":
            args.append(q8)
        elif name == "xv":
            args.append(v8)
        else:
            args.append(consts[name])
    zeros = disp["zeros_fn"]()
    out_arrs = disp["sharded"](*args, zeros)
    out = np.asarray(out_arrs[0]).reshape(B, S, EMBED)
    return out, None


def kernel(**inputs):
    out, _ = run_cores(inputs, trace=False)
    return out


# revision 12
# speedup vs baseline: 2.4003x; 1.1445x over previous
"""HSA (hierarchical splat attention) Bass kernel for Trainium2, 8 NeuronCores.

Math (per batch b):
    q = query @ Wq.T + bq                      [S, D]
    v = value @ Wv.T + bv                      [S, D]
    d2[s,n]  = |q_s|^2 - 2 q_s.c_n + |c_n|^2
    G[s,n]   = exp(-d2[s,n] * inv2v[n]),  inv2v = 0.5*exp(-2*log_scales)
    Asym[s,t]= sum_n G[s,n]*amp[n]*G[t,n]      (rank-N_SPLATS!)
    A        = Asym / (rowsum(Asym) + eps)
    out      = A @ v ;  y = out @ Wo.T + bo

Everything downstream of G is pushed through the rank-64 bottleneck
(G' = G*sqrt(amp), Asym = G'G'^T is never materialized):
    P    = G'^T @ Xv                   [N, D]  (raw values - no v-projection!)
    W2   = P @ (Wv.T Wo.T) + gsum x (bv Wo.T)   [N, D]  (WVO precomputed host-side)
    y[s] = (G'[s,:] @ W2) / (G'[s,:].gsum + eps) + bo
where gsum = G'^T @ 1.  The only full-size GEMM left is the q-projection
(needed for |q_s|^2 inside d2).  The pair AllGather exchanges only P_own+gsum
(bf16 [64,1026]); W2's psum accumulation is split around it (own half before,
peer half after - exact, since peer = (b0+b1) - own is bf16-representable).

Sharding: core c = (batch b = c//2, seq-half h = c%2). Each core receives its
own 1024-token halves of query/value as contiguous f32 views (zero host prep),
PE-transposes Xq on device, and a single pair-wise AllGather of [64,1026] f32
(P_own + per-half gsum) completes the token contraction. Weights/constants are
content-hash cached device-resident arrays, so steady-state host->device
traffic is just the q,v halves in and y out.
"""

import numpy as np
import ml_dtypes

BF16 = ml_dtypes.bfloat16
EMBED = 1024
S = 2048
NSPL = 64
B = 4
NCORES = 8
P = 128
KC = EMBED // P   # 8 chunks over d/e
SOWN = S // 2     # 1024 own tokens per core
SCH = SOWN // P   # 8 own s/t chunks
MW = EMBED + 2    # AllGather payload: 1024 P-cols + 2 gsum half-cols
EPS = 1e-8

_PROG = None       # cached bass program
_DISPATCH = None   # cached jit etc.
_CONSTS = None     # cached (key, device_arrays)


def _build_program():
    import concourse.mybir as mybir
    from concourse import bacc
    from concourse.tile import TileContext
    from concourse.bass import ts, ds

    f32 = mybir.dt.float32
    bf16 = mybir.dt.bfloat16
    AF = mybir.ActivationFunctionType

    nc = bacc.Bacc("TRN2", target_bir_lowering=False, debug=False,
                   num_devices=NCORES)
    xq = nc.declare_dram_parameter("xq", [SOWN, EMBED], f32, isOutput=False)
    xv = nc.declare_dram_parameter("xv", [SOWN, EMBED], f32, isOutput=False)
    wqT = nc.declare_dram_parameter("wqT", [EMBED, EMBED], bf16, isOutput=False)
    wvoT = nc.declare_dram_parameter("wvoT", [EMBED, EMBED], bf16, isOutput=False)
    ctm2 = nc.declare_dram_parameter("ctm2", [EMBED, NSPL], bf16, isOutput=False)
    bq2 = nc.declare_dram_parameter("bq2", [P, KC], f32, isOutput=False)
    bvo64 = nc.declare_dram_parameter("bvo64", [NSPL, EMBED], f32, isOutput=False)
    bob = nc.declare_dram_parameter("bob", [P, EMBED], f32, isOutput=False)
    scn = nc.declare_dram_parameter("scn", [NSPL, 1], f32, isOutput=False)
    bgs = nc.declare_dram_parameter("bgs", [NSPL, 1], f32, isOutput=False)
    one64 = nc.declare_dram_parameter("one64", [P, NSPL], bf16, isOutput=False)
    eyeb = nc.declare_dram_parameter("eyeb", [P, P], bf16, isOutput=False)
    eyef = nc.declare_dram_parameter("eyef", [P, P], f32, isOutput=False)
    y = nc.declare_dram_parameter("y", [SOWN, EMBED], f32, isOutput=True)

    with TileContext(nc) as tc:
        cpool_cm = tc.tile_pool(name="const", bufs=1)
        cpool = cpool_cm.__enter__()
        bq_sb = cpool.tile([P, KC], f32)
        bvo_sb = cpool.tile([NSPL, EMBED], f32)
        bo_sb = cpool.tile([P, EMBED], f32)
        sc_sb = cpool.tile([NSPL, 1], f32)
        bg_sb = cpool.tile([NSPL, 1], f32)
        o64_sb = cpool.tile([P, NSPL], bf16)
        eyeb_sb = cpool.tile([P, P], bf16)
        eyef_sb = cpool.tile([P, P], f32)
        ct_sb = cpool.tile([P, KC, NSPL], bf16)
        gt = cpool.tile([NSPL, SOWN], bf16)     # G'^T own: [n, s_own]
        gT = cpool.tile([P, SCH, NSPL], bf16)   # G' own:   [t_own, n]
        gs_own = cpool.tile([NSPL, 2], f32)     # per-half gsum accum
        xvb = cpool.tile([P, SCH, EMBED], bf16)  # Xv own, natural, bf16

        nc.sync.dma_start(bq_sb[:], bq2[:])
        nc.sync.dma_start(sc_sb[:], scn[:])
        nc.sync.dma_start(bg_sb[:], bgs[:])
        nc.scalar.dma_start(o64_sb[:], one64[:])
        nc.scalar.dma_start(eyeb_sb[:], eyeb[:])
        nc.sync.dma_start(eyef_sb[:], eyef[:])
        ctr = ctm2.rearrange("(h c p) n -> h p c n", p=P, c=4)
        for k in range(2):
            nc.scalar.dma_start(ct_sb[:, k * 4:(k + 1) * 4], ctr[k])
        nc.scalar.dma_start(bvo_sb[:], bvo64[:])
        nc.sync.dma_start(bo_sb[:], bob[:])

        # ---------------- Phase A: q side (load, transpose, project, G') ----
        with tc.tile_pool(name="pa", bufs=1) as pa, \
             tc.tile_pool(name="qe", bufs=3) as qep, \
             tc.tile_pool(name="sqe", bufs=3) as sqp, \
             tc.tile_pool(name="pst", bufs=2, space="PSUM") as pst, \
             tc.tile_pool(name="psq", bufs=4, space="PSUM") as psq, \
             tc.tile_pool(name="psd2", bufs=2, space="PSUM") as psd2:
            wq = pa.tile([P, KC, EMBED], bf16)
            wqr = wqT.rearrange("(h c p) e -> h p c e", p=P, c=4)
            for k in range(2):
                nc.sync.dma_start(wq[:, k * 4:(k + 1) * 4], wqr[k])
            xq_nat = pa.tile([P, SCH, EMBED], f32)
            xqr = xq.rearrange("(g c p) d -> g p c d", p=P, c=2)
            for g in range(4):
                eng = nc.sync if g % 2 == 0 else nc.scalar
                eng.dma_start(xq_nat[:, g * 2:(g + 1) * 2], xqr[g])
            # load + cast Xv early too (needed right after G')
            xv_nat = pa.tile([P, SCH, EMBED], f32)
            xvr = xv.rearrange("(g c p) d -> g p c d", p=P, c=2)
            for g in range(4):
                nc.gpsimd.dma_start(xv_nat[:, g * 2:(g + 1) * 2], xvr[g])
            for c in range(SCH):
                nc.gpsimd.tensor_copy(xvb[:, c], xv_nat[:, c])

            # PE-transpose Xq 128x128 tiles (f32 in, bf16 out via copy)
            xqT = pa.tile([P, KC, SOWN], bf16)
            for dch in range(KC):
                for s2 in range(2):
                    tp = pst.tile([P, 512], f32, tag="tp")
                    for k in range(4):
                        sch = s2 * 4 + k
                        nc.tensor.transpose(
                            tp[:, ts(k, P)],
                            xq_nat[:, sch, ts(dch, P)], eyef_sb[:])
                    if (dch + s2) % 2 == 0:
                        nc.scalar.activation(xqT[:, dch, ts(s2, 512)], tp,
                                             AF.Copy)
                    else:
                        nc.vector.tensor_copy(xqT[:, dch, ts(s2, 512)], tp)

            d2ps = [psd2.tile([NSPL, 512], f32, tag="d2", name=f"d2ps{i}")
                    for i in range(2)]
            for e in range(KC):
                qps = [psq.tile([P, 512], f32, tag="qps", name=f"qps{e}_{i}")
                       for i in range(2)]
                for k in range(KC):
                    for s2 in range(2):
                        nc.tensor.matmul(
                            qps[s2], wq[:, k, ts(e, P)],
                            xqT[:, k, ts(s2, 512)],
                            start=(k == 0), stop=(k == KC - 1))
                qe = qep.tile([P, SOWN], bf16, tag="qe")
                for s2 in range(2):
                    if s2 == 0:
                        nc.scalar.activation(qe[:, ts(s2, 512)], qps[s2],
                                             AF.Identity, bias=bq_sb[:, ds(e, 1)])
                    else:
                        nc.vector.tensor_scalar_add(qe[:, ts(s2, 512)], qps[s2],
                                                    bq_sb[:, ds(e, 1)])
                sq = sqp.tile([P, SOWN], bf16, tag="sq")
                nc.vector.tensor_mul(sq, qe, qe)
                for s2 in range(2):
                    nc.tensor.matmul(d2ps[s2], ct_sb[:, e], qe[:, ts(s2, 512)],
                                     start=(e == 0), stop=False)
                    nc.tensor.matmul(d2ps[s2], o64_sb[:], sq[:, ts(s2, 512)],
                                     start=False, stop=(e == KC - 1))
            # G' = exp(-inv2v*d2 + (-inv2v*c2 + 0.5*ln amp)); accum -> gsum
            for s2 in range(2):
                nc.scalar.activation(gt[:, ts(s2, 512)], d2ps[s2], AF.Exp,
                                     bias=bg_sb[:], scale=sc_sb[:],
                                     accum_out=gs_own[:, ds(s2, 1)])

        # gT = transpose(gt): [t_own, n] chunks
        with tc.tile_pool(name="pstg", bufs=2, space="PSUM") as pstg:
            for tch in range(SCH):
                tp = pstg.tile([P, NSPL], bf16, tag="tpg")
                nc.tensor.transpose(tp[:], gt[:, ts(tch, P)],
                                    eyeb_sb[0:NSPL, 0:NSPL])
                if tch % 2 == 0:
                    nc.vector.tensor_copy(gT[:, tch], tp)
                else:
                    nc.scalar.activation(gT[:, tch], tp, AF.Copy)

        # ---------------- Phase B: P = G'^T Xv, pair AllGather ----------
        # W2 = P @ WVO psum chain spans the collective: own half before,
        # peer half after.
        mpool_cm = tc.tile_pool(name="mpool", bufs=1)
        mpool = mpool_cm.__enter__()
        m_sb = mpool.tile([NSPL, MW], bf16)
        pr_sb = mpool.tile([NSPL, 2, MW], bf16)
        wpool_cm = tc.tile_pool(name="wpool", bufs=1)
        wpool = wpool_cm.__enter__()
        wvo = wpool.tile([P, KC, EMBED], bf16)
        wvor = wvoT.rearrange("(h c p) e -> h p c e", p=P, c=4)
        for k in range(2):
            nc.gpsimd.dma_start(wvo[:, k * 4:(k + 1) * 4], wvor[k])
        w2 = wpool.tile([NSPL, EMBED], bf16)
        rsin = wpool.tile([P, SCH], f32)
        gsc = wpool.tile([NSPL, 1], bf16)
        pT = wpool.tile([P, KC, NSPL], bf16)

        psW_cm = tc.tile_pool(name="psW", bufs=2, space="PSUM")
        psW = psW_cm.__enter__()
        wps = [psW.tile([NSPL, 512], f32, tag="wps", name=f"wps{i}")
               for i in range(2)]
        with tc.tile_pool(name="psP", bufs=2, space="PSUM") as psP, \
             tc.tile_pool(name="psPT", bufs=2, space="PSUM") as psPT, \
             tc.tile_pool(name="dram", bufs=1, space="DRAM") as dram:
            pps = [psP.tile([NSPL, 512], f32, tag="pps", name=f"pps{i}")
                   for i in range(2)]
            for t in range(SCH):
                for mh in range(2):
                    nc.tensor.matmul(pps[mh], gT[:, t],
                                     xvb[:, t, ts(mh, 512)],
                                     start=(t == 0), stop=(t == SCH - 1))
            nc.scalar.activation(m_sb[:, 0:512], pps[0], AF.Copy)
            nc.vector.tensor_copy(m_sb[:, 512:1024], pps[1])
            nc.vector.tensor_copy(m_sb[:, EMBED:MW], gs_own)
            md_in = dram.tile([NSPL, MW], bf16)
            md_out = dram.tile([2, NSPL, MW], bf16)
            nc.sync.dma_start(md_in[:], m_sb[:])
            nc.gpsimd.collective_compute(
                "AllGather", mybir.AluOpType.bypass,
                replica_groups=[[0, 1], [2, 3], [4, 5], [6, 7]],
                ins=[md_in[:].opt()], outs=[md_out[:].opt()])
            # overlap with the collective: PT_own transposes + own W2 half
            for ech in range(KC):
                tp = psPT.tile([P, NSPL], bf16, tag="tpt")
                nc.tensor.transpose(tp[:], m_sb[:, ts(ech, P)],
                                    eyeb_sb[0:NSPL, 0:NSPL])
                if ech % 2 == 0:
                    nc.vector.tensor_copy(pT[:, ech], tp)
                else:
                    nc.scalar.activation(pT[:, ech], tp, AF.Copy)
            for ech in range(KC):
                for eh in range(2):
                    nc.tensor.matmul(wps[eh], pT[:, ech],
                                     wvo[:, ech, ts(eh, 512)],
                                     start=(ech == 0), stop=False)
            mdv = md_out.rearrange("h n w -> n h w")
            nc.sync.dma_start(pr_sb[:], mdv)

        # ---------------- Phase C: peer W2 half, gsum, rs ----------
        with tc.tile_pool(name="pc", bufs=1) as pc, \
             tc.tile_pool(name="psPT2", bufs=2, space="PSUM") as psPT2:
            # peer block = (b0 + b1) - own, exact in bf16
            psum2 = pc.tile([NSPL, MW], f32)
            nc.vector.tensor_add(psum2, pr_sb[:, 0], pr_sb[:, 1])
            p_peer = pc.tile([NSPL, EMBED], bf16)
            nc.vector.tensor_sub(p_peer, psum2[:, 0:EMBED], m_sb[:, 0:EMBED])
            gs2 = pc.tile([NSPL, 2], f32)
            nc.vector.tensor_copy(gs2, psum2[:, EMBED:MW])
            gsum = pc.tile([NSPL, 1], f32)
            nc.vector.tensor_add(gsum, gs2[:, 0:1], gs2[:, 1:2])
            nc.vector.tensor_copy(gsc, gsum)
            pTp = pc.tile([P, KC, NSPL], bf16)
            for ech in range(KC):
                tp = psPT2.tile([P, NSPL], bf16, tag="tpt2")
                nc.tensor.transpose(tp[:], p_peer[:, ts(ech, P)],
                                    eyeb_sb[0:NSPL, 0:NSPL])
                if ech % 2 == 0:
                    nc.vector.tensor_copy(pTp[:, ech], tp)
                else:
                    nc.scalar.activation(pTp[:, ech], tp, AF.Copy)
            for ech in range(KC):
                for eh in range(2):
                    nc.tensor.matmul(wps[eh], pTp[:, ech],
                                     wvo[:, ech, ts(eh, 512)],
                                     start=False, stop=(ech == KC - 1))
            gbv = pc.tile([NSPL, EMBED], f32)
            nc.vector.tensor_scalar_mul(gbv, bvo_sb, gsum)
            for eh in range(2):
                nc.vector.tensor_add(w2[:, ts(eh, 512)], wps[eh],
                                     gbv[:, ts(eh, 512)])
        psW_cm.__exit__(None, None, None)

        # ---------------- Phase D: rs, then y = (G' @ W2) * rsin + bo ------
        with tc.tile_pool(name="ybuf", bufs=3) as yb, \
             tc.tile_pool(name="psrs", bufs=1, space="PSUM") as psrs, \
             tc.tile_pool(name="psy", bufs=3, space="PSUM") as psy:
            rsc = psrs.tile([P, SCH], f32, tag="rsc")
            for sch in range(SCH):
                nc.tensor.matmul(rsc[:, ds(sch, 1)], gt[:, ts(sch, P)],
                                 gsc, start=True, stop=True)
            rst = yb.tile([P, SCH], f32, tag="rst")
            nc.vector.tensor_scalar_add(rst, rsc, EPS)
            nc.vector.reciprocal(rsin, rst)
            yr = y.rearrange("(c p) e -> c p e", p=P)
            for sc in range(SCH):
                yps = psy.tile([P, EMBED], f32, tag="yps")
                for eh in range(2):
                    nc.tensor.matmul(yps[:, ts(eh, 512)], gt[:, ts(sc, P)],
                                     w2[:, ts(eh, 512)], start=True, stop=True)
                yt = yb.tile([P, EMBED], f32, tag="yt")
                nc.scalar.activation(yt, yps, AF.Identity,
                                     scale=rsin[:, ds(sc, 1)])
                ysb = yb.tile([P, EMBED], f32, tag="ysb")
                if sc % 2 == 0:
                    nc.vector.tensor_add(ysb, yt, bo_sb)
                else:
                    nc.gpsimd.tensor_add(ysb, yt, bo_sb)
                eng = nc.sync if sc % 2 == 0 else nc.scalar
                eng.dma_start(yr[sc], ysb)
        wpool_cm.__exit__(None, None, None)
        mpool_cm.__exit__(None, None, None)
        cpool_cm.__exit__(None, None, None)

    nc.finalize()
    return nc


def _const_arrays(Wq, bq, Wv, bv, Wo, bo, C, ls, amp):
    """Host-side constant prep (cached; runs once per weight set)."""
    f = np.float32
    Wq = np.asarray(Wq, f); bq = np.asarray(bq, f)
    Wv = np.asarray(Wv, f); bv = np.asarray(bv, f)
    Wo = np.asarray(Wo, f); bo = np.asarray(bo, f)
    C = np.asarray(C, f); ls = np.asarray(ls, f); amp = np.asarray(amp, f)
    inv2v = 0.5 * np.exp(-2.0 * ls).astype(f)
    c2 = (C.astype(np.float64) ** 2).sum(1)
    wvo = (Wv.T.astype(np.float64) @ Wo.T.astype(np.float64)).astype(f)
    bvo = (bv.astype(np.float64) @ Wo.T.astype(np.float64)).astype(f)
    out = {
        "wqT": np.ascontiguousarray(Wq.T).astype(BF16),
        "wvoT": wvo.astype(BF16),
        "ctm2": np.ascontiguousarray((-2.0 * C).T).astype(BF16),
        "bq2": np.ascontiguousarray(bq.reshape(KC, P).T),
        "bvo64": np.ascontiguousarray(np.broadcast_to(bvo, (NSPL, EMBED))),
        "bob": np.ascontiguousarray(np.broadcast_to(bo, (P, EMBED))),
        "scn": (-inv2v).reshape(NSPL, 1).astype(f),
        # fold sqrt(amp) into G': exp(x + 0.5 ln amp)
        "bgs": (-inv2v * c2 + 0.5 * np.log(np.maximum(amp, 1e-38))
                ).reshape(NSPL, 1).astype(f),
        "one64": np.ones((P, NSPL), BF16),
        "eyeb": np.eye(P, dtype=BF16),
        "eyef": np.eye(P, dtype=np.float32),
    }
    return out


def _weights_key(arrs):
    """Cheap content fingerprint: data pointer + shape + sampled bytes."""
    import hashlib
    h = hashlib.blake2b(digest_size=16)
    for a in arrs:
        a = np.asarray(a)
        ai = a.__array_interface__
        h.update(str((ai["data"][0], a.shape, str(a.dtype))).encode())
        raw = a.reshape(-1)
        step = max(1, raw.size // 4096)
        h.update(np.ascontiguousarray(raw[::step]).tobytes())
    return h.digest()


def _get_dispatch():
    """Build program + jit once; returns dispatch closure state."""
    global _PROG, _DISPATCH
    if _DISPATCH is not None:
        return _DISPATCH
    import jax
    import jax.numpy as jnp
    from jax.sharding import Mesh, PartitionSpec, NamedSharding
    from jax.experimental.shard_map import shard_map
    import concourse.mybir as mybir
    from concourse.bass2jax import (_bass_exec_p, partition_id_tensor,
                                    install_neuronx_cc_hook)

    if _PROG is None:
        _PROG = _build_program()
    nc = _PROG
    install_neuronx_cc_hook()

    in_names = []
    out_names = []
    out_avals = []
    for alloc in nc.m.functions[0].allocations:
        if not isinstance(alloc, mybir.MemoryLocationSet):
            continue
        name = alloc.memorylocations[0].name
        if alloc.kind == "ExternalInput":
            if nc.partition_id_tensor is None or name != nc.partition_id_tensor.name:
                in_names.append(name)
        elif alloc.kind == "ExternalOutput":
            out_names.append(name)
            out_avals.append(jax.core.ShapedArray(
                tuple(alloc.tensor_shape), mybir.dt.np(alloc.dtype)))
    n_params = len(in_names)
    all_names = in_names + out_names
    if nc.partition_id_tensor is not None:
        all_names.append(nc.partition_id_tensor.name)

    def _body(*args):
        operands = list(args)
        if nc.partition_id_tensor is not None:
            operands.append(partition_id_tensor())
        outs = _bass_exec_p.bind(
            *operands, out_avals=tuple(out_avals), in_names=tuple(all_names),
            out_names=tuple(out_names), lowering_input_output_aliases=(),
            sim_require_finite=True, sim_require_nnan=True, nc=nc)
        return tuple(outs)

    devices = jax.devices()[:NCORES]
    mesh = Mesh(np.asarray(devices), ("core",))
    n_outs = len(out_names)
    sharded = jax.jit(
        shard_map(_body, mesh=mesh,
                  in_specs=(PartitionSpec("core"),) * (n_params + n_outs),
                  out_specs=(PartitionSpec("core"),) * n_outs,
                  check_rep=False),
        donate_argnums=tuple(range(n_params, n_params + n_outs)))
    zeros_fn = jax.jit(
        lambda: jnp.zeros((NCORES * SOWN, EMBED), jnp.float32),
        out_shardings=NamedSharding(mesh, PartitionSpec("core")))

    _DISPATCH = dict(in_names=in_names, mesh=mesh, sharded=sharded,
                     zeros_fn=zeros_fn, jax=jax)
    return _DISPATCH


def _get_consts(disp, Wq, bq, Wv, bv, Wo, bo, C, ls, amp):
    """Device-resident constant arrays, cached across calls by content key."""
    global _CONSTS
    ws = (Wq, bq, Wv, bv, Wo, bo, C, ls, amp)
    key = _weights_key(ws)
    if _CONSTS is not None and _CONSTS[0] == key:
        return _CONSTS[1]
    import jax
    from jax.sharding import NamedSharding, PartitionSpec
    host = _const_arrays(*ws)
    dev = {}
    for name, arr in host.items():
        # replicate: stack per-core copies along axis 0 (in_spec P("core"))
        stacked = np.broadcast_to(
            arr, (NCORES,) + arr.shape).reshape(NCORES * arr.shape[0],
                                                *arr.shape[1:])
        dev[name] = jax.device_put(
            np.ascontiguousarray(stacked),
            NamedSharding(disp["mesh"], PartitionSpec("core")))
    _CONSTS = (key, dev)
    return dev


def run_cores(inputs, trace=False):
    """Run the SPMD kernel; returns (full_output, None)."""
    disp = _get_dispatch()
    q = np.asarray(inputs["query"], np.float32)
    v = np.asarray(inputs["value"], np.float32)
    consts = _get_consts(
        disp, inputs["Wq"], inputs["bq"], inputs["Wv"], inputs["bv"],
        inputs["Wo"], inputs["bo"], inputs["splat_centers"],
        inputs["splat_log_scales"], inputs["splat_amplitudes"])
    q8 = np.ascontiguousarray(q).reshape(NCORES * SOWN, EMBED)
    v8 = np.ascontiguousarray(v).reshape(NCORES * SOWN, EMBED)
    args = []
    for name in disp["in_names"]:
        if name == "xq":
            args.append(q8)
        elif name == "xv":
            args.append(v8)
        else:
            args.append(consts[name])
    zeros = disp["zeros_fn"]()
    out_arrs = disp["sharded"](*args, zeros)
    out = np.asarray(out_arrs[0]).reshape(B, S, EMBED)
    return out, None


def kernel(**inputs):
    out, _ = run_cores(inputs, trace=False)
    return out


# revision 14
# speedup vs baseline: 2.5417x; 1.0589x over previous
"""HSA (hierarchical splat attention) Bass kernel for Trainium2, 8 NeuronCores.

Math (per batch b):
    q = query @ Wq.T + bq                      [S, D]
    v = value @ Wv.T + bv                      [S, D]
    d2[s,n]  = |q_s|^2 - 2 q_s.c_n + |c_n|^2
    G[s,n]   = exp(-d2[s,n] * inv2v[n]),  inv2v = 0.5*exp(-2*log_scales)
    Asym[s,t]= sum_n G[s,n]*amp[n]*G[t,n]      (rank-N_SPLATS!)
    A        = Asym / (rowsum(Asym) + eps)
    out      = A @ v ;  y = out @ Wo.T + bo

Everything downstream of G is pushed through the rank-64 bottleneck
(G' = G*sqrt(amp), Asym = G'G'^T is never materialized):
    P    = G'^T @ Xv                   [N, D]  (raw values - no v-projection!)
    W2   = P @ (Wv.T Wo.T) + gsum x (bv Wo.T)   [N, D]  (WVO precomputed host-side)
    y[s] = (G'[s,:] @ W2) / (G'[s,:].gsum + eps) + bo
where gsum = G'^T @ 1.  The only full-size GEMM left is the q-projection
(needed for |q_s|^2 inside d2).  The pair AllGather exchanges only P_own+gsum
(bf16 [64,1026]); W2's psum accumulation is split around it (own half before,
peer half after - exact, since peer = (b0+b1) - own is bf16-representable).

Sharding: core c = (batch b = c//2, seq-half h = c%2). Each core receives its
own 1024-token halves of query/value as contiguous f32 views (zero host prep),
PE-transposes Xq on device, and a single pair-wise AllGather of [64,1026] f32
(P_own + per-half gsum) completes the token contraction. Weights/constants are
content-hash cached device-resident arrays, so steady-state host->device
traffic is just the q,v halves in and y out.
"""

import numpy as np
import ml_dtypes

BF16 = ml_dtypes.bfloat16
EMBED = 1024
S = 2048
NSPL = 64
B = 4
NCORES = 8
P = 128
KC = EMBED // P   # 8 chunks over d/e
SOWN = S // 2     # 1024 own tokens per core
SCH = SOWN // P   # 8 own s/t chunks
MW = EMBED + 2    # AllGather payload: 1024 P-cols + 2 gsum half-cols
EPS = 1e-8

_PROG = None       # cached bass program
_DISPATCH = None   # cached jit etc.
_CONSTS = None     # cached (key, device_arrays)


def _build_program():
    import concourse.mybir as mybir
    from concourse import bacc
    from concourse.tile import TileContext
    from concourse.bass import ts, ds

    f32 = mybir.dt.float32
    bf16 = mybir.dt.bfloat16
    AF = mybir.ActivationFunctionType

    nc = bacc.Bacc("TRN2", target_bir_lowering=False, debug=False,
                   num_devices=NCORES)
    xq = nc.declare_dram_parameter("xq", [SOWN, EMBED], f32, isOutput=False)
    xv = nc.declare_dram_parameter("xv", [SOWN, EMBED], f32, isOutput=False)
    wqT = nc.declare_dram_parameter("wqT", [EMBED, EMBED], bf16, isOutput=False)
    wvoT = nc.declare_dram_parameter("wvoT", [EMBED, EMBED], bf16, isOutput=False)
    ctm2 = nc.declare_dram_parameter("ctm2", [EMBED, NSPL], bf16, isOutput=False)
    bq2 = nc.declare_dram_parameter("bq2", [P, KC], f32, isOutput=False)
    bvo64 = nc.declare_dram_parameter("bvo64", [NSPL, EMBED], f32, isOutput=False)
    scn = nc.declare_dram_parameter("scn", [NSPL, 1], f32, isOutput=False)
    bgs = nc.declare_dram_parameter("bgs", [NSPL, 1], f32, isOutput=False)
    one64 = nc.declare_dram_parameter("one64", [P, NSPL], bf16, isOutput=False)
    eyeb = nc.declare_dram_parameter("eyeb", [P, P], bf16, isOutput=False)
    eyef = nc.declare_dram_parameter("eyef", [P, P], f32, isOutput=False)
    y = nc.declare_dram_parameter("y", [SOWN, EMBED], f32, isOutput=True)

    with TileContext(nc) as tc:
        cpool_cm = tc.tile_pool(name="const", bufs=1)
        cpool = cpool_cm.__enter__()
        bq_sb = cpool.tile([P, KC], f32)
        bvo_sb = cpool.tile([NSPL, EMBED], f32)
        sc_sb = cpool.tile([NSPL, 1], f32)
        bg_sb = cpool.tile([NSPL, 1], f32)
        o64_sb = cpool.tile([P, NSPL], bf16)
        eyeb_sb = cpool.tile([P, P], bf16)
        eyef_sb = cpool.tile([P, P], f32)
        ct_sb = cpool.tile([P, KC, NSPL], bf16)
        gt = cpool.tile([NSPL, SOWN], bf16)     # G'^T own: [n, s_own]
        gT = cpool.tile([P, SCH, NSPL], bf16)   # G' own:   [t_own, n]
        gs_own = cpool.tile([NSPL, 2], f32)     # per-half gsum accum
        xvb = cpool.tile([P, SCH, EMBED], bf16)  # Xv own, natural, bf16

        nc.sync.dma_start(eyef_sb[:], eyef[:])
        nc.scalar.dma_start(eyeb_sb[:], eyeb[:])

        # ---------------- Phase A: q side (load, transpose, project, G') ----
        with tc.tile_pool(name="pa", bufs=1) as pa, \
             tc.tile_pool(name="qe", bufs=3) as qep, \
             tc.tile_pool(name="sqe", bufs=3) as sqp, \
             tc.tile_pool(name="pst", bufs=2, space="PSUM") as pst, \
             tc.tile_pool(name="psq", bufs=4, space="PSUM") as psq, \
             tc.tile_pool(name="psd2", bufs=2, space="PSUM") as psd2:
            xq_nat = pa.tile([P, SCH, EMBED], f32)
            xqr = xq.rearrange("(g c p) d -> g p c d", p=P, c=2)
            for g in range(4):
                eng = nc.sync if g % 2 == 0 else nc.scalar
                eng.dma_start(xq_nat[:, g * 2:(g + 1) * 2], xqr[g])
            wq = pa.tile([P, KC, EMBED], bf16)
            wqr = wqT.rearrange("(h c p) e -> h p c e", p=P, c=4)
            for k in range(2):
                nc.sync.dma_start(wq[:, k * 4:(k + 1) * 4], wqr[k])
            # remaining small consts on ACT behind the xq halves
            nc.scalar.dma_start(bq_sb[:], bq2[:])
            nc.scalar.dma_start(sc_sb[:], scn[:])
            nc.scalar.dma_start(bg_sb[:], bgs[:])
            nc.scalar.dma_start(o64_sb[:], one64[:])
            ctr = ctm2.rearrange("(h c p) n -> h p c n", p=P, c=4)
            for k in range(2):
                nc.scalar.dma_start(ct_sb[:, k * 4:(k + 1) * 4], ctr[k])
            nc.scalar.dma_start(bvo_sb[:], bvo64[:])
            # load + cast Xv via software DGE (Pool) - needed only at P time
            xv_nat = pa.tile([P, SCH, EMBED], f32)
            xvr = xv.rearrange("(g c p) d -> g p c d", p=P, c=2)
            for g in range(4):
                nc.gpsimd.dma_start(xv_nat[:, g * 2:(g + 1) * 2], xvr[g])
            for c in range(SCH):
                nc.gpsimd.tensor_copy(xvb[:, c], xv_nat[:, c])

            # PE-transpose Xq 128x128 tiles (f32 in, bf16 out via copy)
            xqT = pa.tile([P, KC, SOWN], bf16)
            for dch in range(KC):
                for s2 in range(2):
                    tp = pst.tile([P, 512], f32, tag="tp")
                    for k in range(4):
                        sch = s2 * 4 + k
                        nc.tensor.transpose(
                            tp[:, ts(k, P)],
                            xq_nat[:, sch, ts(dch, P)], eyef_sb[:])
                    if (dch + s2) % 2 == 0:
                        nc.scalar.activation(xqT[:, dch, ts(s2, 512)], tp,
                                             AF.Copy)
                    else:
                        nc.vector.tensor_copy(xqT[:, dch, ts(s2, 512)], tp)

            d2ps = [psd2.tile([NSPL, 512], f32, tag="d2", name=f"d2ps{i}")
                    for i in range(2)]
            for e in range(KC):
                qps = [psq.tile([P, 512], f32, tag="qps", name=f"qps{e}_{i}")
                       for i in range(2)]
                for k in range(KC):
                    for s2 in range(2):
                        nc.tensor.matmul(
                            qps[s2], wq[:, k, ts(e, P)],
                            xqT[:, k, ts(s2, 512)],
                            start=(k == 0), stop=(k == KC - 1))
                qe = qep.tile([P, SOWN], bf16, tag="qe")
                for s2 in range(2):
                    if s2 == 0:
                        nc.scalar.activation(qe[:, ts(s2, 512)], qps[s2],
                                             AF.Identity, bias=bq_sb[:, ds(e, 1)])
                    else:
                        nc.vector.tensor_scalar_add(qe[:, ts(s2, 512)], qps[s2],
                                                    bq_sb[:, ds(e, 1)])
                sq = sqp.tile([P, SOWN], bf16, tag="sq")
                nc.vector.tensor_mul(sq, qe, qe)
                for s2 in range(2):
                    nc.tensor.matmul(d2ps[s2], ct_sb[:, e], qe[:, ts(s2, 512)],
                                     start=(e == 0), stop=False)
                    nc.tensor.matmul(d2ps[s2], o64_sb[:], sq[:, ts(s2, 512)],
                                     start=False, stop=(e == KC - 1))
            # G' = exp(-inv2v*d2 + (-inv2v*c2 + 0.5*ln amp)); accum -> gsum
            for s2 in range(2):
                nc.scalar.activation(gt[:, ts(s2, 512)], d2ps[s2], AF.Exp,
                                     bias=bg_sb[:], scale=sc_sb[:],
                                     accum_out=gs_own[:, ds(s2, 1)])

        # gT = transpose(gt): [t_own, n] chunks
        with tc.tile_pool(name="pstg", bufs=2, space="PSUM") as pstg:
            for tch in range(SCH):
                tp = pstg.tile([P, NSPL], bf16, tag="tpg")
                nc.tensor.transpose(tp[:], gt[:, ts(tch, P)],
                                    eyeb_sb[0:NSPL, 0:NSPL])
                if tch % 2 == 0:
                    nc.vector.tensor_copy(gT[:, tch], tp)
                else:
                    nc.scalar.activation(gT[:, tch], tp, AF.Copy)

        # ---------------- Phase B: P = G'^T Xv, pair AllGather ----------
        # W2 = P @ WVO psum chain spans the collective: own half before,
        # peer half after.
        mpool_cm = tc.tile_pool(name="mpool", bufs=1)
        mpool = mpool_cm.__enter__()
        m_sb = mpool.tile([NSPL, MW], bf16)
        pr_sb = mpool.tile([NSPL, 2, MW], bf16)
        wpool_cm = tc.tile_pool(name="wpool", bufs=1)
        wpool = wpool_cm.__enter__()
        wvo = wpool.tile([P, KC, EMBED], bf16)
        wvor = wvoT.rearrange("(h c p) e -> h p c e", p=P, c=4)
        for k in range(2):
            nc.gpsimd.dma_start(wvo[:, k * 4:(k + 1) * 4], wvor[k])
        w2 = wpool.tile([NSPL, EMBED], bf16)
        rsin = wpool.tile([P, SCH], f32)
        gsc = wpool.tile([NSPL, 1], bf16)
        pT = wpool.tile([P, KC, NSPL], bf16)

        psW_cm = tc.tile_pool(name="psW", bufs=2, space="PSUM")
        psW = psW_cm.__enter__()
        wps = [psW.tile([NSPL, 512], f32, tag="wps", name=f"wps{i}")
               for i in range(2)]
        with tc.tile_pool(name="psP", bufs=2, space="PSUM") as psP, \
             tc.tile_pool(name="psPT", bufs=2, space="PSUM") as psPT, \
             tc.tile_pool(name="dram", bufs=1, space="DRAM") as dram:
            pps = [psP.tile([NSPL, 512], f32, tag="pps", name=f"pps{i}")
                   for i in range(2)]
            for t in range(SCH):
                for mh in range(2):
                    nc.tensor.matmul(pps[mh], gT[:, t],
                                     xvb[:, t, ts(mh, 512)],
                                     start=(t == 0), stop=(t == SCH - 1))
            nc.scalar.activation(m_sb[:, 0:512], pps[0], AF.Copy)
            nc.vector.tensor_copy(m_sb[:, 512:1024], pps[1])
            nc.vector.tensor_copy(m_sb[:, EMBED:MW], gs_own)
            md_in = dram.tile([NSPL, MW], bf16)
            md_out = dram.tile([2, NSPL, MW], bf16)
            nc.sync.dma_start(md_in[:], m_sb[:])
            nc.gpsimd.collective_compute(
                "AllGather", mybir.AluOpType.bypass,
                replica_groups=[[0, 1], [2, 3], [4, 5], [6, 7]],
                ins=[md_in[:].opt()], outs=[md_out[:].opt()])
            # overlap with the collective: PT_own transposes + own W2 half
            for ech in range(KC):
                tp = psPT.tile([P, NSPL], bf16, tag="tpt")
                nc.tensor.transpose(tp[:], m_sb[:, ts(ech, P)],
                                    eyeb_sb[0:NSPL, 0:NSPL])
                if ech % 2 == 0:
                    nc.vector.tensor_copy(pT[:, ech], tp)
                else:
                    nc.scalar.activation(pT[:, ech], tp, AF.Copy)
            for ech in range(KC):
                for eh in range(2):
                    nc.tensor.matmul(wps[eh], pT[:, ech],
                                     wvo[:, ech, ts(eh, 512)],
                                     start=(ech == 0), stop=False)
            mdv = md_out.rearrange("h n w -> n h w")
            nc.sync.dma_start(pr_sb[:], mdv)

        # ---------------- Phase C: peer W2 half, gsum, rs ----------
        with tc.tile_pool(name="pc", bufs=1) as pc, \
             tc.tile_pool(name="psPT2", bufs=2, space="PSUM") as psPT2:
            # peer block = (b0 + b1) - own, exact in bf16
            psum2 = pc.tile([NSPL, MW], f32)
            nc.vector.tensor_add(psum2, pr_sb[:, 0], pr_sb[:, 1])
            p_peer = pc.tile([NSPL, EMBED], bf16)
            nc.vector.tensor_sub(p_peer, psum2[:, 0:EMBED], m_sb[:, 0:EMBED])
            gs2 = pc.tile([NSPL, 2], f32)
            nc.vector.tensor_copy(gs2, psum2[:, EMBED:MW])
            gsum = pc.tile([NSPL, 1], f32)
            nc.vector.tensor_add(gsum, gs2[:, 0:1], gs2[:, 1:2])
            nc.vector.tensor_copy(gsc, gsum)
            pTp = pc.tile([P, KC, NSPL], bf16)
            for ech in range(KC):
                tp = psPT2.tile([P, NSPL], bf16, tag="tpt2")
                nc.tensor.transpose(tp[:], p_peer[:, ts(ech, P)],
                                    eyeb_sb[0:NSPL, 0:NSPL])
                if ech % 2 == 0:
                    nc.vector.tensor_copy(pTp[:, ech], tp)
                else:
                    nc.scalar.activation(pTp[:, ech], tp, AF.Copy)
            for ech in range(KC):
                for eh in range(2):
                    nc.tensor.matmul(wps[eh], pTp[:, ech],
                                     wvo[:, ech, ts(eh, 512)],
                                     start=False, stop=(ech == KC - 1))
            gbv = pc.tile([NSPL, EMBED], f32)
            nc.vector.tensor_scalar_mul(gbv, bvo_sb, gsum)
            for eh in range(2):
                nc.vector.tensor_add(w2[:, ts(eh, 512)], wps[eh],
                                     gbv[:, ts(eh, 512)])
        psW_cm.__exit__(None, None, None)

        # ---------------- Phase D: rs, then y = (G' @ W2) * rsin + bo ------
        with tc.tile_pool(name="ybuf", bufs=3) as yb, \
             tc.tile_pool(name="psrs", bufs=1, space="PSUM") as psrs, \
             tc.tile_pool(name="psy", bufs=3, space="PSUM") as psy:
            rsc = psrs.tile([P, SCH], f32, tag="rsc")
            for sch in range(SCH):
                nc.tensor.matmul(rsc[:, ds(sch, 1)], gt[:, ts(sch, P)],
                                 gsc, start=True, stop=True)
            rst = yb.tile([P, SCH], f32, tag="rst")
            nc.vector.tensor_scalar_add(rst, rsc, EPS)
            nc.vector.reciprocal(rsin, rst)
            import concourse.mybir as _mb
            yr = y.rearrange("(c p) e -> c p e", p=P)
            for sc in range(SCH):
                yps = psy.tile([P, EMBED], f32, tag="yps")
                for eh in range(2):
                    nc.tensor.matmul(yps[:, ts(eh, 512)], gt[:, ts(sc, P)],
                                     w2[:, ts(eh, 512)], start=True, stop=True)
                yt = yb.tile([P, EMBED], f32, tag="yt")
                nc.scalar.activation(yt, yps, AF.Identity,
                                     scale=rsin[:, ds(sc, 1)])
                # y output buffer arrives pre-filled with bo (donated input);
                # accumulate-on-write adds it
                nc.gpsimd.dma_start(yr[sc], yt, accum_op=_mb.AluOpType.add)
        wpool_cm.__exit__(None, None, None)
        mpool_cm.__exit__(None, None, None)
        cpool_cm.__exit__(None, None, None)

    nc.finalize()
    return nc


def _const_arrays(Wq, bq, Wv, bv, Wo, bo, C, ls, amp):
    """Host-side constant prep (cached; runs once per weight set)."""
    f = np.float32
    Wq = np.asarray(Wq, f); bq = np.asarray(bq, f)
    Wv = np.asarray(Wv, f); bv = np.asarray(bv, f)
    Wo = np.asarray(Wo, f); bo = np.asarray(bo, f)
    C = np.asarray(C, f); ls = np.asarray(ls, f); amp = np.asarray(amp, f)
    inv2v = 0.5 * np.exp(-2.0 * ls).astype(f)
    c2 = (C.astype(np.float64) ** 2).sum(1)
    wvo = (Wv.T.astype(np.float64) @ Wo.T.astype(np.float64)).astype(f)
    bvo = (bv.astype(np.float64) @ Wo.T.astype(np.float64)).astype(f)
    out = {
        "wqT": np.ascontiguousarray(Wq.T).astype(BF16),
        "wvoT": wvo.astype(BF16),
        "ctm2": np.ascontiguousarray((-2.0 * C).T).astype(BF16),
        "bq2": np.ascontiguousarray(bq.reshape(KC, P).T),
        "bvo64": np.ascontiguousarray(np.broadcast_to(bvo, (NSPL, EMBED))),
        "_bo": bo.copy(),
        "scn": (-inv2v).reshape(NSPL, 1).astype(f),
        # fold sqrt(amp) into G': exp(x + 0.5 ln amp)
        "bgs": (-inv2v * c2 + 0.5 * np.log(np.maximum(amp, 1e-38))
                ).reshape(NSPL, 1).astype(f),
        "one64": np.ones((P, NSPL), BF16),
        "eyeb": np.eye(P, dtype=BF16),
        "eyef": np.eye(P, dtype=np.float32),
    }
    return out


def _weights_key(arrs):
    """Cheap content fingerprint: data pointer + shape + sampled bytes."""
    import hashlib
    h = hashlib.blake2b(digest_size=16)
    for a in arrs:
        a = np.asarray(a)
        ai = a.__array_interface__
        h.update(str((ai["data"][0], a.shape, str(a.dtype))).encode())
        raw = a.reshape(-1)
        step = max(1, raw.size // 4096)
        h.update(np.ascontiguousarray(raw[::step]).tobytes())
    return h.digest()


def _get_dispatch():
    """Build program + jit once; returns dispatch closure state."""
    global _PROG, _DISPATCH
    if _DISPATCH is not None:
        return _DISPATCH
    import jax
    import jax.numpy as jnp
    from jax.sharding import Mesh, PartitionSpec, NamedSharding
    from jax.experimental.shard_map import shard_map
    import concourse.mybir as mybir
    from concourse.bass2jax import (_bass_exec_p, partition_id_tensor,
                                    install_neuronx_cc_hook)

    if _PROG is None:
        _PROG = _build_program()
    nc = _PROG
    install_neuronx_cc_hook()

    in_names = []
    out_names = []
    out_avals = []
    for alloc in nc.m.functions[0].allocations:
        if not isinstance(alloc, mybir.MemoryLocationSet):
            continue
        name = alloc.memorylocations[0].name
        if alloc.kind == "ExternalInput":
            if nc.partition_id_tensor is None or name != nc.partition_id_tensor.name:
                in_names.append(name)
        elif alloc.kind == "ExternalOutput":
            out_names.append(name)
            out_avals.append(jax.core.ShapedArray(
                tuple(alloc.tensor_shape), mybir.dt.np(alloc.dtype)))
    n_params = len(in_names)
    all_names = in_names + out_names
    if nc.partition_id_tensor is not None:
        all_names.append(nc.partition_id_tensor.name)

    def _body(*args):
        operands = list(args)
        if nc.partition_id_tensor is not None:
            operands.append(partition_id_tensor())
        outs = _bass_exec_p.bind(
            *operands, out_avals=tuple(out_avals), in_names=tuple(all_names),
            out_names=tuple(out_names), lowering_input_output_aliases=(),
            sim_require_finite=True, sim_require_nnan=True, nc=nc)
        return tuple(outs)

    devices = jax.devices()[:NCORES]
    mesh = Mesh(np.asarray(devices), ("core",))
    n_outs = len(out_names)
    sharded = jax.jit(
        shard_map(_body, mesh=mesh,
                  in_specs=(PartitionSpec("core"),) * (n_params + n_outs),
                  out_specs=(PartitionSpec("core"),) * n_outs,
                  check_rep=False),
        donate_argnums=tuple(range(n_params, n_params + n_outs)))
    fill_fn = jax.jit(
        lambda b: jnp.broadcast_to(b, (NCORES * SOWN, EMBED)) + 0.0,
        out_shardings=NamedSharding(mesh, PartitionSpec("core")))

    _DISPATCH = dict(in_names=in_names, mesh=mesh, sharded=sharded,
                     fill_fn=fill_fn, jax=jax)
    return _DISPATCH


def _get_consts(disp, Wq, bq, Wv, bv, Wo, bo, C, ls, amp):
    """Device-resident constant arrays, cached across calls by content key."""
    global _CONSTS
    ws = (Wq, bq, Wv, bv, Wo, bo, C, ls, amp)
    key = _weights_key(ws)
    if _CONSTS is not None and _CONSTS[0] == key:
        return _CONSTS[1]
    import jax
    from jax.sharding import NamedSharding, PartitionSpec
    host = _const_arrays(*ws)
    dev = {}
    bo_host = host.pop("_bo")
    dev["_bo"] = jax.device_put(bo_host)
    for name, arr in host.items():
        # replicate: stack per-core copies along axis 0 (in_spec P("core"))
        stacked = np.broadcast_to(
            arr, (NCORES,) + arr.shape).reshape(NCORES * arr.shape[0],
                                                *arr.shape[1:])
        dev[name] = jax.device_put(
            np.ascontiguousarray(stacked),
            NamedSharding(disp["mesh"], PartitionSpec("core")))
    _CONSTS = (key, dev)
    return dev


def run_cores(inputs, trace=False):
    """Run the SPMD kernel; returns (full_output, None)."""
    disp = _get_dispatch()
    q = np.asarray(inputs["query"], np.float32)
    v = np.asarray(inputs["value"], np.float32)
    consts = _get_consts(
        disp, inputs["Wq"], inputs["bq"], inputs["Wv"], inputs["bv"],
        inputs["Wo"], inputs["bo"], inputs["splat_centers"],
        inputs["splat_log_scales"], inputs["splat_amplitudes"])
    q8 = np.ascontiguousarray(q).reshape(NCORES * SOWN, EMBED)
    v8 = np.ascontiguousarray(v).reshape(NCORES * SOWN, EMBED)
    args = []
    for name in disp["in_names"]:
        if name == "xq":
            args.append(q8)
        elif name == "xv":
            args.append(v8)
        else:
            args.append(consts[name])
    fill = disp["fill_fn"](consts["_bo"])
    out_arrs = disp["sharded"](*args, fill)
    out = np.asarray(out_arrs[0]).reshape(B, S, EMBED)
    return out, None


def kernel(**inputs):
    out, _ = run_cores(inputs, trace=False)
    return out
